# revision 1
# baseline (speedup 1.0000x reference)
"""Trainium2 Bass kernel for nn_MessagePassingLayer (graph U-Net message
passing) on 8 NeuronCores.

Self-contained: kernel(**inputs) takes the full unsharded inputs and
returns the full [50000, 128] float32 output.

Strategy: nodes padded to 50176 and sharded contiguously over the 8
cores; edges bucketed per (dst block, src half) in destination-sorted
order. Every per-edge coefficient in this network factorizes as
rowfac[row[e]] * colfac[col[e]], so row factors are pre-applied to the
gathered node table (per-pass XS buffers, exchanged via AllGather) and
col factors are applied per-partition on PSUM eviction. Per pass:
dma_gather of 512B node rows, one-hot is_equal tiles + PE matmul
accumulation per 128-dst block, then W matmul + rank-1 bias + PE
transpose for convs. Top-k pooling runs as an on-device replicated
threshold bisection; degree/weight renormalization factors are computed
with narrow Z-table gather passes.
"""
import math
import os

import numpy as np
import concourse.bacc as bacc
import concourse.mybir as mybir
import concourse.tile as tile
from concourse.bass_utils import run_bass_kernel_spmd

"""Host-side: config, numpy mirror of the reference, static preprocessing."""
import math
from dataclasses import dataclass, field

import numpy as np

P = 128
NCORES = 8


@dataclass
class Cfg:
    N: int = 50000
    E: int = 800000
    D: int = 128
    L: int = 2
    B: int = 2
    ratio: float = 0.5
    # derived
    blocks_per_core: int = field(init=False)
    N_pad: int = field(init=False)
    nodes_per_core: int = field(init=False)
    half: int = field(init=False)          # rows per gather half-table

    def __post_init__(self):
        blocks_total = math.ceil(self.N / P)
        self.blocks_per_core = math.ceil(blocks_total / NCORES)
        self.N_pad = self.blocks_per_core * NCORES * P
        self.nodes_per_core = self.blocks_per_core * P
        self.half = self.N_pad // 2
        assert self.half <= 32768, "gather half-table must fit int16 index"
        assert self.N_pad // 2 % P == 0




def wrap_idx(idx, n):
    """[n] int -> [128, n/16] int16 wrapped+replicated layout for dma_gather."""
    assert n % 16 == 0
    w = np.zeros((16, n // 16), np.int16)
    w[np.arange(n) % 16, np.arange(n) // 16] = idx.astype(np.int16)
    return np.tile(w, (8, 1))


def build_direction(cfg, src, dst, group_blocks):
    """Static tables for one scatter direction.

    src/dst: [E] global endpoint arrays (gather at src, scatter to dst).
    Edges are bucketed per (core, local dst block, src half), padded to
    tiles of 128 with null edges (gather a zero pad row, onehot col 0).
    Tile counts per (block position, half) are maxed across cores so the
    8 cores share one instruction stream.

    Returns dict with per-core arrays + the shared group structure.
    """
    bpc = cfg.blocks_per_core
    npc = cfg.nodes_per_core
    half = cfg.half
    zero_row = cfg.N_pad - 1          # a pad node: always-zero row (half 1)

    # bucket edges
    buckets = [[[None, None] for _ in range(bpc)] for _ in range(NCORES)]
    core_of = dst // npc
    blk_of = (dst % npc) // P
    half_of = (src >= half).astype(np.int64)
    order = np.lexsort((src, half_of, blk_of, core_of))
    so_src, so_dst = src[order], dst[order]
    so_core, so_blk, so_half = core_of[order], blk_of[order], half_of[order]
    # boundaries
    key = ((so_core * bpc) + so_blk) * 2 + so_half
    bounds = np.searchsorted(key, np.arange(NCORES * bpc * 2 + 1))
    for c in range(NCORES):
        for b in range(bpc):
            for h in (0, 1):
                kk = (c * bpc + b) * 2 + h
                s, e = bounds[kk], bounds[kk + 1]
                buckets[c][b][h] = order[s:e]

    # tiles per (block position, half): max over cores
    T = np.zeros((bpc, 2), np.int64)
    for b in range(bpc):
        for h in (0, 1):
            mx = max(len(buckets[c][b][h]) for c in range(NCORES))
            T[b, h] = max(1, math.ceil(mx / P))

    # group structure: consecutive blocks grouped; within a group, halves
    # interleaved per block so each block's psum closes quickly
    groups = []
    for g0 in range(0, bpc, group_blocks):
        groups.append(list(range(g0, min(g0 + group_blocks, bpc))))

    # per-core flattened edge arrays in program order:
    # for group: for h in (0,1): for b in group: T[b,h]*128 edges
    tot_tiles = int(T.sum())
    E_flat = tot_tiles * P
    idxs = np.zeros((NCORES, E_flat), np.int64)
    dstl = np.zeros((NCORES, E_flat), np.float32)
    gvalid = np.zeros(E_flat, bool)  # static: positions with real edges possible
    pos = 0
    tile_plan = []   # (group, h, b, ntiles, start_pos) shared across cores
    for grp in groups:
        for h in (0, 1):
            for b in grp:
                nt = int(T[b, h])
                tile_plan.append((h, b, nt, pos))
                for c in range(NCORES):
                    ed = buckets[c][b][h]
                    n = len(ed)
                    assert n <= nt * P
                    sl = slice(pos, pos + n)
                    idxs[c, sl] = src[ed] - h * half
                    dstl[c, sl] = (dst[ed] % npc) % P
                    # null edges: gather pad zero row (local idx in half 1)
                    if n < nt * P:
                        idxs[c, pos + n : pos + nt * P] = zero_row - half
                gvalid[pos : pos + nt * P] = True
                pos += nt * P
    assert pos == E_flat

    # null edges in half 0 would index out of half range; they were filled
    # with (zero_row - half) which is valid only for h=1 gathers. For h=0
    # tiles, point nulls at local row 0 but with dstl 0 and ZERO gathered
    # contribution required -> instead redirect: null edges always gather
    # half-1 table? Can't: one call per half. Fix: for h=0 null slots use
    # local row 0 and rely on coeff... we don't have per-edge coeffs.
    # Instead: use a dedicated always-zero row in half 0 too: global row
    # cfg.half-1 is a REAL node. So: remap h=0 nulls to the pad zero row of
    # half 0 IF one exists... none guaranteed. Solution: give every h=0
    # null a dstl pointing to a dedicated SCRATCH dst column: impossible.
    # -> Simplest: make h=0 nulls gather real row 0 but mark their onehot
    # column as 128 (out of range) so is_equal never matches: dstl = 128.
    for i, (h, b, nt, p0) in enumerate(tile_plan):
        if h == 0:
            for c in range(NCORES):
                ed = buckets[c][b][h]
                n = len(ed)
                if n < nt * P:
                    idxs[c, p0 + n : p0 + nt * P] = 0
                    dstl[c, p0 + n : p0 + nt * P] = 200.0  # never matches iota
    # also h=1 nulls: dstl 200 for safety (zero row anyway)
    for i, (h, b, nt, p0) in enumerate(tile_plan):
        if h == 1:
            for c in range(NCORES):
                ed = buckets[c][b][h]
                n = len(ed)
                if n < nt * P:
                    dstl[c, p0 + n : p0 + nt * P] = 200.0

    # wrapped idx arrays + dstl in [128, tiles] tile-major layout
    idx16 = np.stack([wrap_idx(idxs[c], E_flat) for c in range(NCORES)])
    ntiles = E_flat // P
    dstl_t = dstl.reshape(NCORES, ntiles, P).transpose(0, 2, 1).copy()

    return {
        "idx16": idx16,            # [NCORES, 128, E_flat/16] int16
        "dstl": dstl_t,            # [NCORES, 128, ntiles] f32
        "tile_plan": tile_plan,    # shared: (h, b, ntiles, start_pos)
        "groups": groups,
        "E_flat": E_flat,
        "ntiles": ntiles,
        "T": T,
    }


def preprocess(cfg, x, edge_index, pvec):
    """All static host work. Returns per-core input maps pieces + meta."""
    N, Np = cfg.N, cfg.N_pad
    row = edge_index[0].astype(np.int64)
    col = edge_index[1].astype(np.int64)
    E = cfg.E

    # static level-0 node quantities (on padded size, pads = 0)
    deg0 = np.zeros(Np, np.float32)
    np.add.at(deg0, row, 1.0)
    dis0 = np.where(deg0 > 0, deg0.astype(np.float64) ** -0.5, 0.0).astype(np.float32)
    normed0 = np.where(deg0 > 0, 1.0 / np.where(deg0 > 0, deg0, 1), 0.0).astype(np.float32)
    A0 = np.zeros(Np, np.float32)
    np.add.at(A0, col, normed0[row])
    aggr0 = (A0 + 1e-12).astype(np.float32)
    r0 = (1.0 / aggr0).astype(np.float32)      # kept_{-1}=1 everywhere real
    r0[deg0 == 0] = 1.0 / 1e-12                # harmless; q0[row]=0 guards
    # NOTE r0 for isolated dst nodes: aggr=1e-12 -> r0 huge but all its
    # incoming w_send are ... wait: aggr0 at node c counts incoming edges;
    # deg0 is outgoing. recompute r0 straight:
    r0 = (1.0 / aggr0).astype(np.float32)
    q0 = normed0
    u0 = dis0
    sigma0 = np.zeros(Np, np.float32)
    np.add.at(sigma0, col, u0[row])

    # directions
    cs = build_direction(cfg, row, col, group_blocks=3)   # gather src=row, scatter dst=col
    rs = build_direction(cfg, col, row, group_blocks=3)   # gather src=col, scatter dst=row

    # initial gather source
    xs0 = np.zeros((Np, cfg.D), np.float32)
    xs0[:N] = x * u0[:N, None]

    # real-node mask (for topk)
    realmask = np.zeros(Np, np.float32)
    realmask[:N] = 1.0

    p0 = pvec[0] / np.linalg.norm(pvec[0])
    p1 = pvec[1] / np.linalg.norm(pvec[1])

    def shardvec(v):   # [Np] -> [NCORES, 128, bpc] (partition-major per block)
        return v.reshape(NCORES, cfg.blocks_per_core, P).transpose(0, 2, 1).copy()

    def fullvec(v):    # [Np] -> [128, blocks_total]
        return v.reshape(-1, P).T.copy()

    meta = {
        "cs": cs, "rs": rs,
        "u0": u0, "q0": q0, "r0": r0, "sigma0": sigma0, "aggr0": aggr0,
        "xs0": xs0, "realmask": realmask, "p0": p0, "p1": p1,
        "shardvec": shardvec, "fullvec": fullvec,
    }
    return meta




F32 = mybir.dt.float32
I16 = mybir.dt.int16
AF = mybir.ActivationFunctionType
OP = mybir.AluOpType
AX = mybir.AxisListType

ZCHUNK = 40
ZW = 64
BISECT_ITERS = 46


class G:
    """build-time globals bag"""
    pass


# ------------------------------------------------------------- edge passes --

def emit_dir_pass(g, dirn, src_dram, mode, block_fn, zcols=0):
    """One edge pass. mode: 'conv' (psum [f,d]), 'wec' (psum [d,f]), 'z'."""
    nc = g.nc
    d = g.dirs[dirn]
    elem = ZW if mode == "z" else g.D
    if mode == "z":
        in_aps = [g.z_dram.ap()[0 : g.half, :], g.z_dram.ap()[g.half :, :]]
    else:
        in_aps = [src_dram.ap()[0 : g.half, :], src_dram.ap()[g.half :, :]]

    plan = {(h, b): (nt, pos) for (h, b, nt, pos) in d["tile_plan"]}
    psums = {}
    done = {}
    for grp in d["groups"]:
        parts = {}
        for h in (0, 1):
            t0 = plan[(h, grp[0])][1] // P
            ntg = sum(plan[(h, b)][0] for b in grp)
            nidx = ntg * P
            it = g.sb_idx.tile([128, nidx // 16], I16, name="idx", tag="idx")
            nc.sync.dma_start(
                out=it[:], in_=d["idx16_d"].ap()[:, t0 * 8 : t0 * 8 + nidx // 16])
            gt = g.sb_gath.tile([P, ntg, elem], F32,
                                name="fg", tag="fg")
            nc.gpsimd.dma_gather(
                out_ap=gt[:], in_ap=in_aps[h], idxs_ap=it[:],
                num_idxs=nidx, num_idxs_reg=nidx, elem_size=elem,
                single_packet=False)
            dl = g.sb_idx.tile([P, ntg], F32, name="dl", tag="dl")
            nc.sync.dma_start(out=dl[:], in_=d["dstl_d"].ap()[:, t0 : t0 + ntg])
            oh = g.sb_oht.tile([P, ntg, P], F32, name="oht", tag="oht")
            nc.vector.tensor_tensor(
                out=oh[:], in0=dl[:, :, None].to_broadcast([P, ntg, P]),
                in1=g.iota_big[:, : ntg, :], op=OP.is_equal)
            parts[h] = (gt, oh, t0)
        for h in (0, 1):
            gt, oh, t0 = parts[h]
            for b in grp:
                nt, pos = plan[(h, b)]
                rel = pos // P - t0
                if b not in psums:
                    pp = [zcols, P] if mode == "z" else [P, P]
                    psums[b] = g.ps_agg.tile(pp, F32, space="PSUM", name="agg", tag="agg")
                    done[b] = 0
                tot = plan[(0, b)][0] + plan[(1, b)][0]
                for t in range(nt):
                    done[b] += 1
                    if mode == "conv":
                        lhs, rhs = gt[:, rel + t, :], oh[:, rel + t, :]
                    elif mode == "wec":
                        lhs, rhs = oh[:, rel + t, :], gt[:, rel + t, :]
                    else:
                        lhs, rhs = gt[:, rel + t, :zcols], oh[:, rel + t, :]
                    nc.tensor.matmul(out=psums[b][:], lhsT=lhs, rhs=rhs,
                                     start=(done[b] == 1), stop=(done[b] == tot))
                    if done[b] == tot:
                        block_fn(b, psums[b])
                        del psums[b], done[b]


def conv_block_fn(g, W_sb, b_sb, sigma_row, outs):
    nc = g.nc

    def fn(b, pag):
        a1 = g.sb_ev.tile([P, P], F32, name="a1", tag="a1")
        nc.vector.tensor_copy(out=a1[:], in_=pag[:])
        p2 = g.ps_w.tile([P, P], F32, space="PSUM", name="p2", tag="p2")
        nc.tensor.matmul(out=p2[:], lhsT=W_sb[:], rhs=a1[:], start=True, stop=False)
        nc.tensor.matmul(out=p2[:], lhsT=b_sb[:],
                         rhs=sigma_row[:, b * P : (b + 1) * P],
                         start=False, stop=True)
        a2 = g.sb_ev.tile([P, P], F32, name="a2", tag="a2")
        nc.vector.tensor_copy(out=a2[:], in_=p2[:])
        p3 = g.ps_t.tile([P, P], F32, space="PSUM", name="pst", tag="pst")
        nc.tensor.transpose(out=p3[:], in_=a2[:], identity=g.ident[:])
        emit_evictions(g, b, p3, outs)
    return fn


def wec_block_fn(g, colfac, outs, score_to=None, pbc=None):
    nc = g.nc

    def fn(b, pag):
        if score_to is not None:
            sc = g.sb_ev.tile([P, P], F32, name="scm", tag="scm")
            nc.vector.tensor_tensor(out=sc[:], in0=pag[:], in1=pbc[:], op=OP.mult)
            red = g.sb_ev.tile([P, 1], F32, name="scr", tag="scr")
            nc.vector.reduce_sum(red[:], sc[:], axis=AX.X)
            nc.vector.tensor_tensor(out=score_to[:, b : b + 1], in0=red[:],
                                    in1=colfac[:, b : b + 1], op=OP.mult)
        emit_evictions(g, b, pag, outs)
    return fn


def z_block_fn(g, row_to, col_to, zcols):
    """row_to: [(rowtile, j)] copy psum row j; col_to: [(coltile, j)]."""
    nc = g.nc

    def fn(b, pag):
        az = g.sb_ev.tile([P, P], F32, name="az", tag="az")
        nc.vector.tensor_copy(out=az[:zcols, :], in_=pag[:])
        for (rt, j) in row_to:
            nc.vector.tensor_copy(out=rt[:, b * P : (b + 1) * P],
                                  in_=az[j : j + 1, :])
        if col_to:
            pz = g.ps_t.tile([P, P], F32, space="PSUM", name="pst", tag="pst")
            nc.tensor.transpose(out=pz[:, :zcols], in_=az[:zcols, :],
                                identity=g.ident[:zcols, :zcols])
            for (ct, j) in col_to:
                nc.vector.tensor_copy(out=ct[:, b : b + 1],
                                      in_=pz[:, j : j + 1])
    return fn


def emit_evictions(g, b, psum, outs):
    nc = g.nc
    for o in outs:
        kind = o[0]
        if kind == "xs":
            _, dram, scalevec = o
            t = g.sb_out.tile([P, P], F32, name="xso", tag="xso")
            nc.scalar.activation(out=t[:], in_=psum[:], func=AF.Copy,
                                 scale=scalevec[:, b : b + 1])
            nc.sync.dma_start(
                out=dram.ap().rearrange("(b p) d -> b p d", p=P)[b], in_=t[:])
        elif kind == "addshard":
            _, dram_in, dram_out, pre, post = o
            t = g.sb_out.tile([P, P], F32, name="aso", tag="aso")
            nc.scalar.activation(out=t[:], in_=psum[:], func=AF.Copy,
                                 scale=pre[:, b : b + 1])
            sk = g.sb_out.tile([P, P], F32, name="skl", tag="skl")
            nc.sync.dma_start(
                out=sk[:], in_=dram_in.ap().rearrange("(b p) d -> b p d", p=P)[b])
            nc.vector.tensor_tensor(out=t[:], in0=t[:], in1=sk[:], op=OP.add)
            if post is not None:
                nc.vector.tensor_scalar(out=t[:], in0=t[:],
                                        scalar1=post[:, b : b + 1], scalar2=None,
                                        op0=OP.mult)
            nc.sync.dma_start(
                out=dram_out.ap().rearrange("(b p) d -> b p d", p=P)[b], in_=t[:])


# ------------------------------------------------------------- small pieces --

def allgather(g, in_dram, out_dram):
    g.nc.gpsimd.collective_compute(
        "AllGather", OP.bypass, replica_groups=[list(range(NCORES))],
        ins=[in_dram.ap()], outs=[out_dram.ap()])


def zbuild(g, cols):
    nc = g.nc
    for c0 in range(0, g.BT, ZCHUNK):
        nb = min(ZCHUNK, g.BT - c0)
        st = g.sb_zst.tile([P, ZCHUNK, ZW], F32, name="zst", tag="zst")
        for j, v in enumerate(cols):
            nc.vector.tensor_copy(out=st[:, :nb, j : j + 1],
                                  in_=v[:, c0 : c0 + nb, None])
        nc.sync.dma_start(
            out=g.z_dram.ap().rearrange("(b p) w -> p b w", p=P)[:, c0 : c0 + nb, :],
            in_=st[:, :nb, :])


def cross_part(g, col, op):
    nc = g.nc
    if op == "sum":
        pc = g.ps_t.tile([P, P], F32, space="PSUM", name="pst", tag="pst")
        nc.tensor.matmul(out=pc[:1, :1], lhsT=col[:], rhs=g.ones_col[:],
                         start=True, stop=True)
        out = g.sb_bis.tile([1, 1], F32, name="cnt", tag="cnt")
        nc.vector.tensor_copy(out=out[:], in_=pc[:1, :1])
        return out
    pt = g.ps_t.tile([P, P], F32, space="PSUM", name="pst", tag="pst")
    nc.tensor.transpose(out=pt[:1, :], in_=col[:], identity=g.ident[:])
    row = g.sb_bis.tile([1, P], F32, name="brow", tag="brow")
    nc.vector.tensor_copy(out=row[:], in_=pt[:1, :])
    out = g.sb_bis.tile([1, 1], F32, name="bred", tag="bred")
    nc.vector.reduce_max(out[:], row[:], axis=AX.X)
    return out


def bcast_scalar(g, s11, tag):
    nc = g.nc
    pb = g.ps_t.tile([P, P], F32, space="PSUM", name="pst", tag="pst")
    nc.tensor.matmul(out=pb[:, :1], lhsT=g.ones_row[:], rhs=s11[:],
                     start=True, stop=True)
    out = g.sb_nv.tile([P, 1], F32, name=tag, tag=tag)
    nc.vector.tensor_copy(out=out[:], in_=pb[:, :1])
    return out


def bisect_topk(g, sel_full, k, tag):
    """threshold col [128,1] such that count(sel >= t) == k exactly."""
    nc = g.nc
    mx = g.sb_bis.tile([P, 1], F32, name="bmx", tag="bmx")
    nc.vector.reduce_max(mx[:], sel_full[:], axis=AX.X)
    hi = cross_part(g, mx, "max")
    nc.vector.tensor_scalar(out=hi[:], in0=hi[:], scalar1=1.0, scalar2=None,
                            op0=OP.add)
    lo = g.sb_bis.tile([1, 1], F32, name="blo", tag="blo")
    nc.vector.tensor_scalar(out=lo[:], in0=hi[:], scalar1=-4e4, scalar2=None,
                            op0=OP.add)
    t = g.sb_bis.tile([1, 1], F32, name="bt", tag="bt")
    for _ in range(BISECT_ITERS):
        nc.vector.tensor_tensor(out=t[:], in0=lo[:], in1=hi[:], op=OP.add)
        nc.vector.tensor_scalar(out=t[:], in0=t[:], scalar1=0.5, scalar2=None,
                                op0=OP.mult)
        tcol = bcast_scalar(g, t, "btc")
        cmp = g.sb_bis.tile([P, g.BT], F32, name="bcmp", tag="bcmp")
        nc.vector.tensor_scalar(out=cmp[:], in0=sel_full[:], scalar1=tcol[:],
                                scalar2=None, op0=OP.is_ge)
        red = g.sb_bis.tile([P, 1], F32, name="bred2", tag="bred2")
        nc.vector.reduce_sum(red[:], cmp[:], axis=AX.X)
        cnt = cross_part(g, red, "sum")
        flag = g.sb_bis.tile([1, 1], F32, name="bflag", tag="bflag")
        nc.vector.tensor_scalar(out=flag[:], in0=cnt[:], scalar1=float(k) - 0.5,
                                scalar2=None, op0=OP.is_ge)
        d1 = g.sb_bis.tile([1, 1], F32, name="bd1", tag="bd1")
        nc.vector.tensor_tensor(out=d1[:], in0=t[:], in1=lo[:], op=OP.subtract)
        nc.vector.tensor_tensor(out=d1[:], in0=d1[:], in1=flag[:], op=OP.mult)
        nc.vector.tensor_tensor(out=lo[:], in0=lo[:], in1=d1[:], op=OP.add)
        nf = g.sb_bis.tile([1, 1], F32, name="bnf", tag="bnf")
        nc.vector.tensor_scalar(out=nf[:], in0=flag[:], scalar1=-1.0, scalar2=1.0,
                                op0=OP.mult, op1=OP.add)
        d2 = g.sb_bis.tile([1, 1], F32, name="bd2", tag="bd2")
        nc.vector.tensor_tensor(out=d2[:], in0=t[:], in1=hi[:], op=OP.subtract)
        nc.vector.tensor_tensor(out=d2[:], in0=d2[:], in1=nf[:], op=OP.mult)
        nc.vector.tensor_tensor(out=hi[:], in0=hi[:], in1=d2[:], op=OP.add)
    return bcast_scalar(g, lo, tag)


def load_full_from_ag(g, ag_dram, tag, nvec=1, vec=0):
    """AG out dram [(8*nvec*128), bpc] -> [128, BT] sbuf."""
    nc = g.nc
    out = g.sb_nv.tile([P, g.BT], F32, name=tag, tag=tag)
    for r in range(NCORES):
        src = ag_dram.ap().rearrange("(r v p) b -> r v p b", v=nvec, p=P)[r, vec]
        nc.sync.dma_start(out=out[:, r * g.bpc : (r + 1) * g.bpc], in_=src)
    return out


def nv(g, tag, shape=None):
    return g.sb_nv.tile(shape or [P, g.bpc], F32, name=tag, tag=tag)


def sel_from(g, score, active, tag):
    """sel = score*active + (active-1)*1e30 (elementwise, any width)."""
    nc = g.nc
    t1 = nv(g, tag, [P, score.shape[-1]])
    nc.vector.tensor_tensor(out=t1[:], in0=score[:], in1=active[:], op=OP.mult)
    t2 = nv(g, tag + "_m", [P, score.shape[-1]])
    nc.vector.tensor_scalar(out=t2[:], in0=active[:], scalar1=1e30,
                            scalar2=-1e30, op0=OP.mult, op1=OP.add)
    nc.vector.tensor_tensor(out=t1[:], in0=t1[:], in1=t2[:], op=OP.add)
    return t1


import numpy as np
import concourse.bacc as bacc
import concourse.mybir as mybir
import concourse.tile as tile
from concourse.bass_utils import run_bass_kernel_spmd



def build_kernel(cfg, meta):
    g = G()
    g.D = cfg.D
    g.half = cfg.half
    g.bpc = cfg.blocks_per_core
    g.BT = cfg.N_pad // P
    npc = cfg.nodes_per_core
    Np = cfg.N_pad

    nc = bacc.Bacc(trn_type="TRN2")
    g.nc = nc

    cs, rs = meta["cs"], meta["rs"]
    maxtg = 0
    for d in (cs, rs):
        for grp in d["groups"]:
            for h in (0, 1):
                maxtg = max(maxtg, sum(d["T"][b][h] for b in grp))
    g.maxtg = int(maxtg)

    # ---- params
    def par(name, shape, dt=F32):
        return nc.declare_dram_parameter(name, list(shape), dt, isOutput=False)

    xs0p = par("xs0", [Np, cfg.D])
    g.dirs = {}
    for nm, d in (("cs", cs), ("rs", rs)):
        g.dirs[nm] = dict(d)
        g.dirs[nm]["idx16_d"] = par(f"idx16_{nm}", [128, d["E_flat"] // 16], I16)
        g.dirs[nm]["dstl_d"] = par(f"dstl_{nm}", [128, d["ntiles"]])
    statc = par("statc", [P, g.bpc, 6])     # u0,q0,r0,aggr0,realmask_sh,pad
    sig0p = par("sigma0", [1, npc])
    rmfp = par("realmask_full", [P, g.BT])
    iotap = par("iotabig", [P, g.maxtg, P])
    identp = par("ident", [P, P])
    onesp = par("ones", [P, 2])             # col of ones; col 1 unused
    pbcp = par("pbc", [2 * P, P])           # p0,p1 broadcast tiles
    wallp = par("wall", [10 * P, P])
    ballp = par("ball", [1, 10 * P])
    out_p = nc.declare_dram_parameter("out", [npc, cfg.D], F32, isOutput=True)

    # ---- internal dram
    def dram(name, shape, shared=False):
        return nc.dram_tensor(name, list(shape), F32,
                              addr_space="Shared" if shared else "Local")

    XS = {k: dram(f"xs{k}", [Np, cfg.D], shared=True) for k in range(1, 14)}
    xsout = {k: dram(f"xso{k}", [npc, cfg.D]) for k in range(1, 14)}
    g.z_dram = dram("ztab", [Np, ZW])
    h2save = dram("h2save", [npc, cfg.D])
    h5save = dram("h5save", [npc, cfg.D])
    h3tmp = dram("h3tmp", [npc, cfg.D])
    h6tmp = dram("h6tmp", [npc, cfg.D])
    score_sh_d = {i: dram(f"scsh{i}", [P, g.bpc]) for i in (0, 1)}
    score_fl_d = {i: dram(f"scfl{i}", [NCORES * P, g.bpc], shared=True) for i in (0, 1)}
    uq_sh_d = dram("uqsh", [2 * P, g.bpc])
    uq_fl_d = dram("uqfl", [NCORES * 2 * P, g.bpc], shared=True)
    u2_sh_d = dram("u2sh", [P, g.bpc])
    u2_fl_d = dram("u2fl", [NCORES * P, g.bpc], shared=True)

    with tile.TileContext(nc) as tc:
        g.tc = tc
        ctxs = [
            tc.tile_pool(name="const", bufs=1),
            tc.tile_pool(name="nvp", bufs=1),
            tc.tile_pool(name="idxp", bufs=2),
            tc.tile_pool(name="gathp", bufs=2),
            tc.tile_pool(name="ohtp", bufs=2),
            tc.tile_pool(name="evp", bufs=3),
            tc.tile_pool(name="outp", bufs=3),
            tc.tile_pool(name="zstp", bufs=1),
            tc.tile_pool(name="bisp", bufs=1),
            tc.tile_pool(name="psagg", bufs=4, space="PSUM"),
            tc.tile_pool(name="psw", bufs=2, space="PSUM"),
            tc.tile_pool(name="pst", bufs=2, space="PSUM"),
        ]
        cpool, g.sb_nv, g.sb_idx, g.sb_gath, g.sb_oht, g.sb_ev, g.sb_out, \
            g.sb_zst, g.sb_bis, g.ps_agg, g.ps_w, g.ps_t = \
            [c.__enter__() for c in ctxs]

        # ---- constants into sbuf
        def cload(ap_src, shape, tag):
            t = cpool.tile(list(shape), F32, name=tag, tag=tag)
            nc.sync.dma_start(out=t[:], in_=ap_src)
            return t

        g.iota_big = cload(iotap.ap(), [P, g.maxtg, P], "iota")
        g.ident = cload(identp.ap(), [P, P], "ident")
        ones2 = cload(onesp.ap(), [P, 2], "ones2")
        g.ones_col = ones2[:, 0:1]
        orow = cpool.tile([1, P], F32, name="orow", tag="orow")
        nc.vector.memset(orow[:], 1.0)
        g.ones_row = orow
        statc_t = cload(statc.ap(), [P, g.bpc, 6], "statc")
        u0c = statc_t[:, :, 0]
        q0c = statc_t[:, :, 1]
        r0c = statc_t[:, :, 2]
        aggr0c = statc_t[:, :, 3]
        rm_sh = statc_t[:, :, 4]
        sig0 = cload(sig0p.ap(), [1, npc], "sig0")
        rm_fl = cload(rmfp.ap(), [P, g.BT], "rmfl")
        pbc_t = cload(pbcp.ap().rearrange("(v p) d -> p v d", p=P), [P, 2, P], "pbc")
        p0bc, p1bc = pbc_t[:, 0, :], pbc_t[:, 1, :]
        wall = cload(wallp.ap().rearrange("(w p) d -> p w d", p=P), [P, 10, P], "wall")
        ball = cload(ballp.ap().rearrange("o (w d) -> o w d", d=P), [1, 10, P], "ball")
        Wt = [wall[:, i, :] for i in range(10)]
        bt = [ball[:, i, :] for i in range(10)]

        # precombined eviction scale vectors (col form [P, bpc])
        def vmul(a, b_, tag):
            t = nv(g, tag)
            nc.vector.tensor_tensor(out=t[:], in0=a[:], in1=b_[:], op=OP.mult)
            return t

        u0u0 = vmul(u0c, u0c, "u0u0")
        u0q0 = vmul(u0c, q0c, "u0q0")

        def conv_pass(widx, src, sigma_row, outs):
            emit_dir_pass(g, "cs", src, "conv",
                            conv_block_fn(g, Wt[widx], bt[widx], sigma_row, outs))

        def emit_schedule():
            # =========== full pass schedule ===========
            conv_pass(0, xs0p, sig0, [("xs", xsout[1], u0u0)])
            allgather(g, xsout[1], XS[1])
            conv_pass(1, XS[1], sig0, [("xs", xsout[2], u0q0), ("xs", h2save, u0c)])
            allgather(g, xsout[2], XS[2])
            # P3 wec + score0
            score0 = nv(g, "score0")
            emit_dir_pass(g, "cs", XS[2], "wec",
                            wec_block_fn(g, r0c, [("xs", h3tmp, r0c)],
                                           score_to=score0, pbc=p0bc))
            nc.sync.dma_start(out=score_sh_d[0].ap(), in_=score0[:])
            allgather(g, score_sh_d[0], score_fl_d[0])
            sc0f = load_full_from_ag(g, score_fl_d[0], "sc0f")
            sel0f = sel_from(g, sc0f, rm_fl, "sel0f")
            k0 = math.ceil(cfg.ratio * cfg.N)
            thr0 = bisect_topk(g, sel0f, k0, "thr0")
            kept0f = nv(g, "kept0f", [P, g.BT])
            nc.vector.tensor_scalar(out=kept0f[:], in0=sel0f[:], scalar1=thr0[:],
                                    scalar2=None, op0=OP.is_ge)
            sel0s = sel_from(g, score0, rm_sh, "sel0s")
            kept0s = nv(g, "kept0s")
            nc.vector.tensor_scalar(out=kept0s[:], in0=sel0s[:], scalar1=thr0[:],
                                    scalar2=None, op0=OP.is_ge)
            tanh0 = nv(g, "tanh0")
            nc.scalar.activation(out=tanh0[:], in_=score0[:], func=AF.Tanh)
            # Z pass A: deg1 raw (rs direction, gather kept0 at col, segsum by row)
            zbuild(g, [kept0f])
            S1 = nv(g, "S1")
            emit_dir_pass(g, "rs", None, "z",
                            z_block_fn(g, [], [(S1, 0)], 1), zcols=1)
            # u1, q1 (shard)
            deg1 = vmul(kept0s, S1, "deg1")
            m1 = nv(g, "m1")
            nc.vector.tensor_scalar(out=m1[:], in0=deg1[:], scalar1=0.0, scalar2=None,
                                    op0=OP.is_gt)
            dsafe = nv(g, "dsafe")
            nc.vector.tensor_scalar(out=dsafe[:], in0=deg1[:], scalar1=1e-30,
                                    scalar2=None, op0=OP.max)
            u1 = nv(g, "u1")
            nc.vector.reciprocal(out=u1[:], in_=dsafe[:])
            nc.scalar.activation(out=u1[:], in_=u1[:], func=AF.Sqrt)
            nc.vector.tensor_tensor(out=u1[:], in0=u1[:], in1=m1[:], op=OP.mult)
            w1 = vmul(aggr0c, kept0s, "w1")
            rdeg1 = nv(g, "rdeg1")
            nc.vector.reciprocal(out=rdeg1[:], in_=dsafe[:])
            q1 = vmul(w1, rdeg1, "q1")
            nc.vector.tensor_tensor(out=q1[:], in0=q1[:], in1=m1[:], op=OP.mult)
            # AG u1,q1
            nc.sync.dma_start(out=uq_sh_d.ap()[0:P], in_=u1[:])
            nc.sync.dma_start(out=uq_sh_d.ap()[P:], in_=q1[:])
            allgather(g, uq_sh_d, uq_fl_d)
            u1f = load_full_from_ag(g, uq_fl_d, "u1f", nvec=2, vec=0)
            q1f = load_full_from_ag(g, uq_fl_d, "q1f", nvec=2, vec=1)
            # Z pass B: sigma1 (row), A1 (col)  (cs direction, gather at row)
            zbuild(g, [u1f, q1f])
            sig1 = cpool.tile([1, npc], F32, name="sig1", tag="sig1")
            A1 = nv(g, "A1")
            emit_dir_pass(g, "cs", None, "z",
                            z_block_fn(g, [(sig1, 0)], [(A1, 1)], 2), zcols=2)
            aggr1 = vmul(kept0s, A1, "aggr1")
            nc.vector.tensor_scalar(out=aggr1[:], in0=aggr1[:], scalar1=1e-12,
                                    scalar2=None, op0=OP.add)
            raggr1 = nv(g, "raggr1")
            nc.vector.reciprocal(out=raggr1[:], in_=aggr1[:])
            r1 = vmul(kept0s, raggr1, "r1")
            # XS3 = h3 * tanh0 * u1
            cv3 = vmul(tanh0, u1, "cv3")
            for b in range(g.bpc):
                t = g.sb_out.tile([P, P], F32, name="rs3", tag="rs3")
                nc.sync.dma_start(
                    out=t[:], in_=h3tmp.ap().rearrange("(b p) d -> b p d", p=P)[b])
                nc.vector.tensor_scalar(out=t[:], in0=t[:], scalar1=cv3[:, b : b + 1],
                                        scalar2=None, op0=OP.mult)
                nc.sync.dma_start(
                    out=xsout[3].ap().rearrange("(b p) d -> b p d", p=P)[b], in_=t[:])
            allgather(g, xsout[3], XS[3])

            # =========== DOWN LEVEL 1 ===========
            u1u1 = vmul(u1, u1, "u1u1")
            u1q1 = vmul(u1, q1, "u1q1")
            conv_pass(2, XS[3], sig1, [("xs", xsout[4], u1u1)])
            allgather(g, xsout[4], XS[4])
            conv_pass(3, XS[4], sig1, [("xs", xsout[5], u1q1), ("xs", h5save, u1)])
            allgather(g, xsout[5], XS[5])
            score1 = nv(g, "score1")
            emit_dir_pass(g, "cs", XS[5], "wec",
                            wec_block_fn(g, r1, [("xs", h6tmp, r1)],
                                           score_to=score1, pbc=p1bc))
            nc.sync.dma_start(out=score_sh_d[1].ap(), in_=score1[:])
            allgather(g, score_sh_d[1], score_fl_d[1])
            sc1f = load_full_from_ag(g, score_fl_d[1], "sc1f")
            sel1f = sel_from(g, sc1f, kept0f, "sel1f")
            k1 = math.ceil(cfg.ratio * k0)
            thr1 = bisect_topk(g, sel1f, k1, "thr1")
            kept1f = nv(g, "kept1f", [P, g.BT])
            nc.vector.tensor_scalar(out=kept1f[:], in0=sel1f[:], scalar1=thr1[:],
                                    scalar2=None, op0=OP.is_ge)
            sel1s = sel_from(g, score1, kept0s, "sel1s")
            kept1s = nv(g, "kept1s")
            nc.vector.tensor_scalar(out=kept1s[:], in0=sel1s[:], scalar1=thr1[:],
                                    scalar2=None, op0=OP.is_ge)
            tanh1 = nv(g, "tanh1")
            nc.scalar.activation(out=tanh1[:], in_=score1[:], func=AF.Tanh)
            # Z pass C: deg2 raw
            zbuild(g, [kept1f])
            S2 = nv(g, "S2")
            emit_dir_pass(g, "rs", None, "z",
                            z_block_fn(g, [], [(S2, 0)], 1), zcols=1)
            deg2 = vmul(kept1s, S2, "deg2")
            m2 = nv(g, "m2")
            nc.vector.tensor_scalar(out=m2[:], in0=deg2[:], scalar1=0.0, scalar2=None,
                                    op0=OP.is_gt)
            d2safe = nv(g, "d2safe")
            nc.vector.tensor_scalar(out=d2safe[:], in0=deg2[:], scalar1=1e-30,
                                    scalar2=None, op0=OP.max)
            u2 = nv(g, "u2")
            nc.vector.reciprocal(out=u2[:], in_=d2safe[:])
            nc.scalar.activation(out=u2[:], in_=u2[:], func=AF.Sqrt)
            nc.vector.tensor_tensor(out=u2[:], in0=u2[:], in1=m2[:], op=OP.mult)
            nc.sync.dma_start(out=u2_sh_d.ap(), in_=u2[:])
            allgather(g, u2_sh_d, u2_fl_d)
            u2f = load_full_from_ag(g, u2_fl_d, "u2f")
            # Z pass D: sigma2 (row only)
            zbuild(g, [u2f])
            sig2 = cpool.tile([1, npc], F32, name="sig2", tag="sig2")
            emit_dir_pass(g, "cs", None, "z",
                            z_block_fn(g, [(sig2, 0)], [], 1), zcols=1)
            # XS6 = h6 * tanh1 * u2
            cv6 = vmul(tanh1, u2, "cv6")
            for b in range(g.bpc):
                t = g.sb_out.tile([P, P], F32, name="rs6", tag="rs6")
                nc.sync.dma_start(
                    out=t[:], in_=h6tmp.ap().rearrange("(b p) d -> b p d", p=P)[b])
                nc.vector.tensor_scalar(out=t[:], in0=t[:], scalar1=cv6[:, b : b + 1],
                                        scalar2=None, op0=OP.mult)
                nc.sync.dma_start(
                    out=xsout[6].ap().rearrange("(b p) d -> b p d", p=P)[b], in_=t[:])
            allgather(g, xsout[6], XS[6])

            # =========== BOTTOM ===========
            u2u2 = vmul(u2, u2, "u2u2")
            u2r1 = vmul(u2, r1, "u2r1")
            conv_pass(4, XS[6], sig2, [("xs", xsout[7], u2u2)])
            allgather(g, xsout[7], XS[7])
            conv_pass(5, XS[7], sig2, [("xs", xsout[8], u2r1)])
            allgather(g, xsout[8], XS[8])

            # =========== UP LEVEL (uses emask1): wec-up + 2 convs ===========
            q1u1 = vmul(q1, u1, "q1u1")
            emit_dir_pass(g, "rs", XS[8], "wec",
                            wec_block_fn(g, q1, [("xs", xsout[9], q1u1)]))
            allgather(g, xsout[9], XS[9])
            conv_pass(6, XS[9], sig1, [("xs", xsout[10], u1u1)])
            allgather(g, xsout[10], XS[10])
            conv_pass(7, XS[10], sig1,
                      [("addshard", h5save, xsout[11], u1, r0c)])
            allgather(g, xsout[11], XS[11])

            # =========== UP LEVEL (emask0) ===========
            q0u0 = vmul(q0c, u0c, "q0u0")
            emit_dir_pass(g, "rs", XS[11], "wec",
                            wec_block_fn(g, q0c, [("xs", xsout[12], q0u0)]))
            allgather(g, xsout[12], XS[12])
            conv_pass(8, XS[12], sig0, [("xs", xsout[13], u0u0)])
            allgather(g, xsout[13], XS[13])
            conv_pass(9, XS[13], sig0,
                      [("addshard", h2save, out_p, u0c, None)])

        emit_schedule()

        for c in reversed(ctxs):
            c.__exit__(None, None, None)

    nc.compile()
    return nc


def make_inmaps(cfg, meta):
    cs, rs = meta["cs"], meta["rs"]
    bpc = cfg.blocks_per_core
    npc = cfg.nodes_per_core
    sv = meta["shardvec"]
    u0s, q0s, r0s, ag0s = (sv(meta[k]) for k in ("u0", "q0", "r0", "aggr0"))
    rms = sv(meta["realmask"])
    sig0s = meta["sigma0"].reshape(NCORES, npc)
    rmf = meta["fullvec"](meta["realmask"])
    maxtg = 0
    for d in (cs, rs):
        for grp in d["groups"]:
            for h in (0, 1):
                maxtg = max(maxtg, sum(d["T"][b][h] for b in grp))
    iota = np.tile(np.arange(P, dtype=np.float32)[None, None, :], (P, maxtg, 1))
    ident = np.eye(P, dtype=np.float32)
    ones = np.ones((P, 2), np.float32)
    pbc = np.concatenate([
        np.tile(meta["p0"][None, :], (P, 1)),
        np.tile(meta["p1"][None, :], (P, 1))], 0).astype(np.float32)

    in_maps = []
    for c in range(NCORES):
        statcv = np.zeros((P, bpc, 6), np.float32)
        statcv[:, :, 0] = u0s[c]
        statcv[:, :, 1] = q0s[c]
        statcv[:, :, 2] = r0s[c]
        statcv[:, :, 3] = ag0s[c]
        statcv[:, :, 4] = rms[c]
        in_maps.append({
            "xs0": meta["xs0"],
            "idx16_cs": cs["idx16"][c], "dstl_cs": cs["dstl"][c],
            "idx16_rs": rs["idx16"][c], "dstl_rs": rs["dstl"][c],
            "statc": statcv, "sigma0": sig0s[c][None, :],
            "realmask_full": rmf,
            "iotabig": iota, "ident": ident, "ones": ones, "pbc": pbc,
            "wall": None, "ball": None,   # filled by caller
        })
    return in_maps


def fill_weights(in_maps, Wd, bd, Wu, bu, Wb, bb):
    Ws = [Wd[0, 0], Wd[0, 1], Wd[1, 0], Wd[1, 1], Wb[0], Wb[1],
          Wu[0, 0], Wu[0, 1], Wu[1, 0], Wu[1, 1]]
    bs = [bd[0, 0], bd[0, 1], bd[1, 0], bd[1, 1], bb[0], bb[1],
          bu[0, 0], bu[0, 1], bu[1, 0], bu[1, 1]]
    wall = np.concatenate([w.astype(np.float32) for w in Ws], 0)
    ball = np.stack([b.astype(np.float32) for b in bs], 0)
    for m in in_maps:
        m["wall"] = wall
        m["ball"] = ball.reshape(1, -1)


def run_gnn(cfg, inputs, nc_cache={}, full_pad=False, trace=False):
    """Full pipeline: preprocess, build (cached by cfg), run, assemble."""
    x = np.asarray(inputs["x"], np.float32)
    ei = np.asarray(inputs["edge_index"])
    pvec = np.asarray(inputs["pvec"], np.float32)
    meta = preprocess(cfg, x, ei, pvec)
    key = (cfg.N, cfg.E, ei.tobytes()[:64])  # program depends on edge stats
    if key not in nc_cache:
        nc_cache.clear()
        nc_cache[key] = (build_kernel(cfg, meta), None)
    nc, _ = nc_cache[key]
    in_maps = make_inmaps(cfg, meta)
    fill_weights(in_maps, *(np.asarray(inputs[k], np.float32)
                            for k in ("Wd", "bd", "Wu", "bu", "Wb", "bb")))
    res = run_bass_kernel_spmd(nc, in_maps, list(range(NCORES)), trace=trace)
    out = np.concatenate([res.results[c]["out"] for c in range(NCORES)], 0)
    return (out if full_pad else out[: cfg.N]), res


_CFG = Cfg()


def kernel(**inputs):
    out, _ = run_gnn(_CFG, inputs)
    return out.astype(np.float32)



# revision 10
# speedup vs baseline: 1.4985x; 1.4985x over previous
"""Trainium2 Bass kernel for nn_MessagePassingLayer (graph U-Net message
passing) on 8 NeuronCores.

Self-contained: kernel(**inputs) takes the full unsharded inputs and
returns the full [50000, 128] float32 output.

Strategy: nodes padded to 50176 and sharded contiguously over the 8
cores; edges bucketed per (dst block, src half) in destination-sorted
order. Every per-edge coefficient in this network factorizes as
rowfac[row[e]] * colfac[col[e]], so row factors are pre-applied to the
gathered node table (per-pass XS buffers, exchanged via AllGather) and
col factors are applied per-partition on PSUM eviction.

Perf structure (v2):
 - dma_gather calls round-robin over 4 SWDGE queues (desc gen runs on
   distinct Q7 core pairs in parallel).
 - passes P7..P14 (bottom + up path, after both top-k selections) run
   with bf16 gather tables and bf16 matmuls; P1..P6 stay f32 so the
   top-k thresholds match the reference bit-for-bit.
 - the sigma1/A1 and sigma2 z-passes are folded into P4/P7 as extra
   gathered columns + a second per-tile matmul into a spare PSUM
   region; only the two deg z-passes (rs direction) remain.
 - trailing padding slots of each gather call use idx=-1 (descriptor
   emission skipped by the Q7 ucode).
"""
import math
import os

import numpy as np
import ml_dtypes
import concourse.bacc as bacc
import concourse.mybir as mybir
import concourse.tile as tile
from concourse.bass_utils import run_bass_kernel_spmd

from dataclasses import dataclass, field

P = 128
NCORES = 8
NQ = 4            # SWDGE queues used round-robin for gathers


@dataclass
class Cfg:
    N: int = 50000
    E: int = 800000
    D: int = 128
    L: int = 2
    B: int = 2
    ratio: float = 0.5
    # derived
    blocks_per_core: int = field(init=False)
    N_pad: int = field(init=False)
    nodes_per_core: int = field(init=False)
    half: int = field(init=False)          # rows per gather half-table

    def __post_init__(self):
        blocks_total = math.ceil(self.N / P)
        self.blocks_per_core = math.ceil(blocks_total / NCORES)
        self.N_pad = self.blocks_per_core * NCORES * P
        self.nodes_per_core = self.blocks_per_core * P
        self.half = self.N_pad // 2
        assert self.half <= 32768, "gather half-table must fit int16 index"
        assert self.N_pad // 2 % P == 0


def wrap_idx(idx, n):
    """[n] int -> [128, n/16] int16 wrapped+replicated layout for dma_gather."""
    assert n % 16 == 0
    w = np.zeros((16, n // 16), np.int16)
    w[np.arange(n) % 16, np.arange(n) // 16] = idx.astype(np.int16)
    return np.tile(w, (8, 1))


def build_direction(cfg, src, dst, group_blocks):
    """Static tables for one scatter direction.

    src/dst: [E] global endpoint arrays (gather at src, scatter to dst).
    Edges are bucketed per (core, local dst block, src half), padded to
    tiles of 128 with null edges. Tile counts per (block position, half)
    are maxed across cores so the 8 cores share one instruction stream.
    Null slots that end up at the tail of a gather call get idx=-1 (the
    gather ucode skips trailing negative indices); interior nulls gather
    the always-zero pad row.
    """
    bpc = cfg.blocks_per_core
    npc = cfg.nodes_per_core
    half = cfg.half
    zero_row = cfg.N_pad - 1          # a pad node: always-zero row (half 1)

    buckets = [[[None, None] for _ in range(bpc)] for _ in range(NCORES)]
    core_of = dst // npc
    blk_of = (dst % npc) // P
    half_of = (src >= half).astype(np.int64)
    order = np.lexsort((src, half_of, blk_of, core_of))
    key = ((core_of[order] * bpc) + blk_of[order]) * 2 + half_of[order]
    bounds = np.searchsorted(key, np.arange(NCORES * bpc * 2 + 1))
    for c in range(NCORES):
        for b in range(bpc):
            for h in (0, 1):
                kk = (c * bpc + b) * 2 + h
                s, e = bounds[kk], bounds[kk + 1]
                buckets[c][b][h] = order[s:e]

    T = np.zeros((bpc, 2), np.int64)
    for b in range(bpc):
        for h in (0, 1):
            mx = max(len(buckets[c][b][h]) for c in range(NCORES))
            T[b, h] = max(1, math.ceil(mx / P))

    groups = []
    for g0 in range(0, bpc, group_blocks):
        groups.append(list(range(g0, min(g0 + group_blocks, bpc))))

    tot_tiles = int(T.sum())
    E_flat = tot_tiles * P
    idxs = np.zeros((NCORES, E_flat), np.int64)
    dstl = np.zeros((NCORES, E_flat), np.float32)
    pos = 0
    tile_plan = []   # (group, h, b, ntiles, start_pos) shared across cores
    for grp in groups:
        for h in (0, 1):
            for gi, b in enumerate(grp):
                nt = int(T[b, h])
                last_in_call = gi == len(grp) - 1
                tile_plan.append((h, b, nt, pos))
                for c in range(NCORES):
                    ed = buckets[c][b][h]
                    n = len(ed)
                    assert n <= nt * P
                    sl = slice(pos, pos + n)
                    idxs[c, sl] = src[ed] - h * half
                    dstl[c, sl] = (dst[ed] % npc) % P
                    if n < nt * P:
                        psl = slice(pos + n, pos + nt * P)
                        dstl[c, psl] = 200.0       # never matches iota
                        idxs[c, psl] = (zero_row - half) if h == 1 else 0
                pos += nt * P
    assert pos == E_flat

    idx16 = np.stack([wrap_idx(idxs[c], E_flat) for c in range(NCORES)])
    ntiles = E_flat // P
    dstl_t = dstl.reshape(NCORES, ntiles, P).transpose(0, 2, 1).copy()

    return {
        "idx16": idx16,            # [NCORES, 128, E_flat/16] int16
        "dstl": dstl_t,            # [NCORES, 128, ntiles] f32
        "tile_plan": tile_plan,    # shared: (h, b, ntiles, start_pos)
        "groups": groups,
        "E_flat": E_flat,
        "ntiles": ntiles,
        "T": T,
    }


def preprocess(cfg, x, edge_index, pvec):
    """All static host work. Returns per-core input maps pieces + meta."""
    N, Np = cfg.N, cfg.N_pad
    row = edge_index[0].astype(np.int64)
    col = edge_index[1].astype(np.int64)

    deg0 = np.zeros(Np, np.float32)
    np.add.at(deg0, row, 1.0)
    with np.errstate(divide="ignore"):
        dis0 = np.where(deg0 > 0, deg0.astype(np.float64) ** -0.5, 0.0
                        ).astype(np.float32)
        normed0 = np.where(deg0 > 0, 1.0 / np.where(deg0 > 0, deg0, 1), 0.0
                           ).astype(np.float32)
    A0 = np.zeros(Np, np.float32)
    np.add.at(A0, col, normed0[row])
    aggr0 = (A0 + 1e-12).astype(np.float32)
    r0 = (1.0 / aggr0).astype(np.float32)
    q0 = normed0
    u0 = dis0
    sigma0 = np.zeros(Np, np.float32)
    np.add.at(sigma0, col, u0[row])

    cs = build_direction(cfg, row, col, group_blocks=2)
    rs = build_direction(cfg, col, row, group_blocks=2)

    xs0 = np.zeros((Np, cfg.D), np.float32)
    xs0[:N] = x * u0[:N, None]

    realmask = np.zeros(Np, np.float32)
    realmask[:N] = 1.0

    p0 = pvec[0] / np.linalg.norm(pvec[0])
    p1 = pvec[1] / np.linalg.norm(pvec[1])

    def shardvec(v):   # [Np] -> [NCORES, 128, bpc] (partition-major per block)
        return v.reshape(NCORES, cfg.blocks_per_core, P).transpose(0, 2, 1).copy()

    def fullvec(v):    # [Np] -> [128, blocks_total]
        return v.reshape(-1, P).T.copy()

    meta = {
        "cs": cs, "rs": rs,
        "u0": u0, "q0": q0, "r0": r0, "sigma0": sigma0, "aggr0": aggr0,
        "xs0": xs0, "realmask": realmask, "p0": p0, "p1": p1,
        "shardvec": shardvec, "fullvec": fullvec,
    }
    return meta


F32 = mybir.dt.float32
BF16 = mybir.dt.bfloat16
I16 = mybir.dt.int16
AF = mybir.ActivationFunctionType
OP = mybir.AluOpType
AX = mybir.AxisListType

ZCHUNK = 40
ZW = 64
BISECT_ITERS = 46


class G:
    """build-time globals bag"""
    pass


# ------------------------------------------------------------- edge passes --

def emit_dir_pass(g, dirn, src_dram, mode, block_fn, zcols=0,
                  elem=128, dt=F32, zmm=0):
    """One edge pass.

    mode: 'conv' (psum [f,d]), 'wec' (psum [d,f]), 'z' (psum [zcols,d]).
    elem: gathered row width in dt elements; cols [P, P+zmm) are per-src
    scalars accumulated into psum region [0:zmm, P:2P] (conv mode only).
    """
    nc = g.nc
    d = g.dirs[dirn]
    if mode == "z":
        elem = ZW
        in_aps = [g.z_dram.ap()[0 : g.half, :], g.z_dram.ap()[g.half :, :]]
    else:
        in_aps = [src_dram.ap()[0 : g.half, :], src_dram.ap()[g.half :, :]]

    plan = {(h, b): (nt, pos) for (h, b, nt, pos) in d["tile_plan"]}
    psums = {}
    done = {}
    for grp in d["groups"]:
        parts = {}
        for h in (0, 1):
            t0 = plan[(h, grp[0])][1] // P
            ntg = sum(plan[(h, b)][0] for b in grp)
            nidx = ntg * P
            it = g.sb_idx.tile([128, nidx // 16], I16, name="idx", tag="idx")
            nc.sync.dma_start(
                out=it[:], in_=d["idx16_d"].ap()[:, t0 * 8 : t0 * 8 + nidx // 16])
            gt = g.sb_gath.tile([P, ntg, elem], dt, name="fg", tag="fg")
            nc.gpsimd.dma_gather(
                out_ap=gt[:], in_ap=in_aps[h], idxs_ap=it[:],
                num_idxs=nidx, num_idxs_reg=nidx, elem_size=elem,
                single_packet=False, queue_num=g.qctr % NQ)
            g.qctr += 1
            dl = g.sb_idx.tile([P, ntg], F32, name="dl", tag="dl")
            nc.sync.dma_start(out=dl[:], in_=d["dstl_d"].ap()[:, t0 : t0 + ntg])
            oh = g.sb_oht.tile([P, ntg, P], dt, name="oht", tag="oht")
            nc.vector.tensor_tensor(
                out=oh[:], in0=dl[:, :, None].to_broadcast([P, ntg, P]),
                in1=g.iota_big[:, : ntg, :], op=OP.is_equal)
            parts[h] = (gt, oh, t0)
        for h in (0, 1):
            gt, oh, t0 = parts[h]
            for b in grp:
                nt, pos = plan[(h, b)]
                rel = pos // P - t0
                if b not in psums:
                    if mode == "z":
                        pp = [zcols, P]
                    elif zmm:
                        pp = [P, 2 * P]
                    else:
                        pp = [P, P]
                    psums[b] = g.ps_agg.tile(pp, F32, space="PSUM", name="agg",
                                             tag="agg")
                    done[b] = 0
                tot = plan[(0, b)][0] + plan[(1, b)][0]
                for t in range(nt):
                    done[b] += 1
                    first, last = done[b] == 1, done[b] == tot
                    if mode == "conv":
                        nc.tensor.matmul(out=psums[b][:, 0:P],
                                         lhsT=gt[:, rel + t, 0:P],
                                         rhs=oh[:, rel + t, :],
                                         start=first, stop=last)
                        if zmm:
                            nc.tensor.matmul(out=psums[b][0:zmm, P : 2 * P],
                                             lhsT=gt[:, rel + t, P : P + zmm],
                                             rhs=oh[:, rel + t, :],
                                             start=first, stop=last)
                    elif mode == "wec":
                        nc.tensor.matmul(out=psums[b][:, 0:P],
                                         lhsT=oh[:, rel + t, :],
                                         rhs=gt[:, rel + t, 0:P],
                                         start=first, stop=last)
                    else:
                        nc.tensor.matmul(out=psums[b][:],
                                         lhsT=gt[:, rel + t, :zcols],
                                         rhs=oh[:, rel + t, :],
                                         start=first, stop=last)
                    if last:
                        block_fn(b, psums[b])
                        del psums[b], done[b]


def conv_block_fn(g, W_sb, b_bc, outs, dt=F32, sigma_col=None, zinfo=None):
    """Per-block eviction for conv passes.

    Bias is applied on DVE at eviction: out += b_bc * (sigma[d] * scale[d]).
    sigma_col: [P, bpc] per-dst sigma shard (used when zinfo is None).
    zinfo: dict(zc=n, cols=[(col_tile, j), ...], bias_col=(tile, j)) --
    sigma/z data come from the pass's own PSUM z region [0:zc, P:2P],
    transposed per block into column tiles.
    """
    nc = g.nc
    ident = g.ident if dt == F32 else g.ident_bf

    def fn(b, pag):
        a1 = g.sb_ev.tile([P, P], dt, name="a1", tag="a1")
        nc.vector.tensor_copy(out=a1[:], in_=pag[:, 0:P])
        if zinfo is not None:
            zc = zinfo["zc"]
            sigz = g.sb_ev.tile([2, P], dt, name="sigz", tag="sigz")
            nc.vector.tensor_copy(out=sigz[:zc, :], in_=pag[0:zc, P : 2 * P])
            pz = g.ps_t.tile([P, P], dt, space="PSUM", name="pst", tag="pst")
            nc.tensor.transpose(out=pz[:, 0:zc], in_=sigz[0:zc, :],
                                identity=ident[0:zc, 0:zc])
            for (ct, j) in zinfo["cols"]:
                nc.vector.tensor_copy(out=ct[:, b : b + 1], in_=pz[:, j : j + 1])
            bt, bj = zinfo["bias_col"]
            bias_col = bt[:, b : b + 1]
        else:
            bias_col = sigma_col[:, b : b + 1]
        p2 = g.ps_w.tile([P, P], F32, space="PSUM", name="p2", tag="p2")
        nc.tensor.matmul(out=p2[:], lhsT=W_sb[:], rhs=a1[:], start=True,
                         stop=True)
        a2 = g.sb_ev.tile([P, P], dt, name="a2", tag="a2")
        nc.vector.tensor_copy(out=a2[:], in_=p2[:])
        p3 = g.ps_t.tile([P, P], dt, space="PSUM", name="pst", tag="pst")
        nc.tensor.transpose(out=p3[:], in_=a2[:], identity=ident[:])
        emit_evictions(g, b, p3, outs, bias_bc=b_bc, bias_col=bias_col)
    return fn


def wec_block_fn(g, colfac, outs, score_to=None, pbc=None):
    nc = g.nc

    def fn(b, pag):
        if score_to is not None:
            sc = g.sb_ev.tile([P, P], F32, name="scm", tag="scm")
            nc.vector.tensor_tensor(out=sc[:], in0=pag[:, 0:P], in1=pbc[:],
                                    op=OP.mult)
            red = g.sb_ev.tile([P, 1], F32, name="scr", tag="scr")
            nc.vector.reduce_sum(red[:], sc[:], axis=AX.X)
            nc.vector.tensor_tensor(out=score_to[:, b : b + 1], in0=red[:],
                                    in1=colfac[:, b : b + 1], op=OP.mult)
        emit_evictions(g, b, pag, outs)
    return fn


def z_block_fn(g, row_to, col_to, zcols):
    """row_to: [(rowtile, j)] copy psum row j; col_to: [(coltile, j)]."""
    nc = g.nc

    def fn(b, pag):
        az = g.sb_ev.tile([P, P], F32, name="az", tag="az")
        nc.vector.tensor_copy(out=az[:zcols, :], in_=pag[:])
        for (rt, j) in row_to:
            nc.vector.tensor_copy(out=rt[:, b * P : (b + 1) * P],
                                  in_=az[j : j + 1, :])
        if col_to:
            pz = g.ps_t.tile([P, P], F32, space="PSUM", name="pst", tag="pst")
            nc.tensor.transpose(out=pz[:, :zcols], in_=az[:zcols, :],
                                identity=g.ident[:zcols, :zcols])
            for (ct, j) in col_to:
                nc.vector.tensor_copy(out=ct[:, b : b + 1],
                                      in_=pz[:, j : j + 1])
    return fn


def emit_evictions(g, b, psum, outs, bias_bc=None, bias_col=None):
    nc = g.nc
    for o in outs:
        kind = o[0]
        if kind == "xs":
            _, dram, scalevec = o
            dt_out = dram.dtype
            t = g.sb_out.tile([P, P], F32, name="xso", tag="xso")
            nc.scalar.activation(out=t[:], in_=psum[:, 0:P], func=AF.Copy,
                                 scale=scalevec[:, b : b + 1])
            to = g.sb_out.tile([P, P], dt_out, name="xso2", tag="xso2")
            if bias_bc is not None:
                sc2 = g.sb_out.tile([P, 1], F32, name="sc2", tag="sc2")
                nc.vector.tensor_tensor(out=sc2[:], in0=bias_col[:],
                                        in1=scalevec[:, b : b + 1], op=OP.mult)
                bt2 = g.sb_out.tile([P, P], F32, name="bt2", tag="bt2")
                nc.vector.tensor_scalar(out=bt2[:], in0=bias_bc[:],
                                        scalar1=sc2[:], scalar2=None,
                                        op0=OP.mult)
                nc.vector.tensor_tensor(out=to[:], in0=t[:], in1=bt2[:],
                                        op=OP.add)
            else:
                nc.vector.tensor_copy(out=to[:], in_=t[:])
            nc.sync.dma_start(
                out=dram.ap().rearrange("(b p) d -> b p d", p=P)[b], in_=to[:])
        elif kind == "addshard":
            _, dram_in, dram_out, pre, post = o
            dt_out = dram_out.dtype
            t = g.sb_out.tile([P, P], F32, name="aso", tag="aso")
            nc.scalar.activation(out=t[:], in_=psum[:, 0:P], func=AF.Copy,
                                 scale=pre[:, b : b + 1])
            if bias_bc is not None:
                sc2 = g.sb_out.tile([P, 1], F32, name="sc2", tag="sc2")
                nc.vector.tensor_tensor(out=sc2[:], in0=bias_col[:],
                                        in1=pre[:, b : b + 1], op=OP.mult)
                bt2 = g.sb_out.tile([P, P], F32, name="bt2", tag="bt2")
                nc.vector.tensor_scalar(out=bt2[:], in0=bias_bc[:],
                                        scalar1=sc2[:], scalar2=None,
                                        op0=OP.mult)
                nc.vector.tensor_tensor(out=t[:], in0=t[:], in1=bt2[:],
                                        op=OP.add)
            sk = g.sb_out.tile([P, P], F32, name="skl", tag="skl")
            nc.sync.dma_start(
                out=sk[:], in_=dram_in.ap().rearrange("(b p) d -> b p d", p=P)[b])
            to = g.sb_out.tile([P, P], dt_out, name="aso2", tag="aso2")
            if post is not None:
                nc.vector.tensor_tensor(out=t[:], in0=t[:], in1=sk[:], op=OP.add)
                nc.vector.tensor_scalar(out=to[:], in0=t[:],
                                        scalar1=post[:, b : b + 1], scalar2=None,
                                        op0=OP.mult)
            else:
                nc.vector.tensor_tensor(out=to[:], in0=t[:], in1=sk[:], op=OP.add)
            nc.sync.dma_start(
                out=dram_out.ap().rearrange("(b p) d -> b p d", p=P)[b], in_=to[:])


# ------------------------------------------------------------- small pieces --

def allgather(g, in_dram, out_dram):
    g.nc.gpsimd.collective_compute(
        "AllGather", OP.bypass, replica_groups=[list(range(NCORES))],
        ins=[in_dram.ap()], outs=[out_dram.ap()])


def zbuild(g, cols):
    nc = g.nc
    for c0 in range(0, g.BT, ZCHUNK):
        nb = min(ZCHUNK, g.BT - c0)
        st = g.sb_zst.tile([P, ZCHUNK, ZW], F32, name="zst", tag="zst")
        for j, v in enumerate(cols):
            nc.vector.tensor_copy(out=st[:, :nb, j : j + 1],
                                  in_=v[:, c0 : c0 + nb, None])
        nc.sync.dma_start(
            out=g.z_dram.ap().rearrange("(b p) w -> p b w", p=P)[:, c0 : c0 + nb, :],
            in_=st[:, :nb, :])


def cross_part(g, col, op):
    nc = g.nc
    if op == "sum":
        pc = g.ps_t.tile([P, P], F32, space="PSUM", name="pst", tag="pst")
        nc.tensor.matmul(out=pc[:1, :1], lhsT=col[:], rhs=g.ones_col[:],
                         start=True, stop=True)
        out = g.sb_bis.tile([1, 1], F32, name="cnt", tag="cnt")
        nc.vector.tensor_copy(out=out[:], in_=pc[:1, :1])
        return out
    pt = g.ps_t.tile([P, P], F32, space="PSUM", name="pst", tag="pst")
    nc.tensor.transpose(out=pt[:1, :], in_=col[:], identity=g.ident[:])
    row = g.sb_bis.tile([1, P], F32, name="brow", tag="brow")
    nc.vector.tensor_copy(out=row[:], in_=pt[:1, :])
    out = g.sb_bis.tile([1, 1], F32, name="bred", tag="bred")
    nc.vector.reduce_max(out[:], row[:], axis=AX.X)
    return out


def bcast_scalar(g, s11, tag):
    nc = g.nc
    pb = g.ps_t.tile([P, P], F32, space="PSUM", name="pst", tag="pst")
    nc.tensor.matmul(out=pb[:, :1], lhsT=g.ones_row[:], rhs=s11[:],
                     start=True, stop=True)
    out = g.sb_nv.tile([P, 1], F32, name=tag, tag=tag)
    nc.vector.tensor_copy(out=out[:], in_=pb[:, :1])
    return out


def bisect_topk(g, sel_full, k, tag):
    """threshold col [128,1] such that count(sel >= t) == k exactly."""
    nc = g.nc
    mx = g.sb_bis.tile([P, 1], F32, name="bmx", tag="bmx")
    nc.vector.reduce_max(mx[:], sel_full[:], axis=AX.X)
    hi = cross_part(g, mx, "max")
    nc.vector.tensor_scalar(out=hi[:], in0=hi[:], scalar1=1.0, scalar2=None,
                            op0=OP.add)
    lo = g.sb_bis.tile([1, 1], F32, name="blo", tag="blo")
    nc.vector.tensor_scalar(out=lo[:], in0=hi[:], scalar1=-4e4, scalar2=None,
                            op0=OP.add)
    t = g.sb_bis.tile([1, 1], F32, name="bt", tag="bt")
    for _ in range(BISECT_ITERS):
        nc.vector.tensor_tensor(out=t[:], in0=lo[:], in1=hi[:], op=OP.add)
        nc.vector.tensor_scalar(out=t[:], in0=t[:], scalar1=0.5, scalar2=None,
                                op0=OP.mult)
        tcol = bcast_scalar(g, t, "btc")
        cmp = g.sb_bis.tile([P, g.BT], F32, name="bcmp", tag="bcmp")
        nc.vector.tensor_scalar(out=cmp[:], in0=sel_full[:], scalar1=tcol[:],
                                scalar2=None, op0=OP.is_ge)
        red = g.sb_bis.tile([P, 1], F32, name="bred2", tag="bred2")
        nc.vector.reduce_sum(red[:], cmp[:], axis=AX.X)
        cnt = cross_part(g, red, "sum")
        flag = g.sb_bis.tile([1, 1], F32, name="bflag", tag="bflag")
        nc.vector.tensor_scalar(out=flag[:], in0=cnt[:], scalar1=float(k) - 0.5,
                                scalar2=None, op0=OP.is_ge)
        d1 = g.sb_bis.tile([1, 1], F32, name="bd1", tag="bd1")
        nc.vector.tensor_tensor(out=d1[:], in0=t[:], in1=lo[:], op=OP.subtract)
        nc.vector.tensor_tensor(out=d1[:], in0=d1[:], in1=flag[:], op=OP.mult)
        nc.vector.tensor_tensor(out=lo[:], in0=lo[:], in1=d1[:], op=OP.add)
        nf = g.sb_bis.tile([1, 1], F32, name="bnf", tag="bnf")
        nc.vector.tensor_scalar(out=nf[:], in0=flag[:], scalar1=-1.0, scalar2=1.0,
                                op0=OP.mult, op1=OP.add)
        d2 = g.sb_bis.tile([1, 1], F32, name="bd2", tag="bd2")
        nc.vector.tensor_tensor(out=d2[:], in0=t[:], in1=hi[:], op=OP.subtract)
        nc.vector.tensor_tensor(out=d2[:], in0=d2[:], in1=nf[:], op=OP.mult)
        nc.vector.tensor_tensor(out=hi[:], in0=hi[:], in1=d2[:], op=OP.add)
    return bcast_scalar(g, lo, tag)


def load_full_from_ag(g, ag_dram, tag, nvec=1, vec=0):
    """AG out dram [(8*nvec*128), bpc] -> [128, BT] sbuf."""
    nc = g.nc
    out = g.sb_nv.tile([P, g.BT], F32, name=tag, tag=tag)
    for r in range(NCORES):
        src = ag_dram.ap().rearrange("(r v p) b -> r v p b", v=nvec, p=P)[r, vec]
        nc.sync.dma_start(out=out[:, r * g.bpc : (r + 1) * g.bpc], in_=src)
    return out


def nv(g, tag, shape=None):
    return g.sb_nv.tile(shape or [P, g.bpc], F32, name=tag, tag=tag)


def sel_from(g, score, active, tag):
    """sel = score*active + (active-1)*1e30 (elementwise, any width)."""
    nc = g.nc
    t1 = nv(g, tag, [P, score.shape[-1]])
    nc.vector.tensor_tensor(out=t1[:], in0=score[:], in1=active[:], op=OP.mult)
    t2 = nv(g, tag + "_m", [P, score.shape[-1]])
    nc.vector.tensor_scalar(out=t2[:], in0=active[:], scalar1=1e30,
                            scalar2=-1e30, op0=OP.mult, op1=OP.add)
    nc.vector.tensor_tensor(out=t1[:], in0=t1[:], in1=t2[:], op=OP.add)
    return t1


def build_kernel(cfg, meta):
    g = G()
    g.D = cfg.D
    g.half = cfg.half
    g.bpc = cfg.blocks_per_core
    g.BT = cfg.N_pad // P
    g.qctr = 0
    npc = cfg.nodes_per_core
    Np = cfg.N_pad

    nc = bacc.Bacc(trn_type="TRN2", num_swdge_queues=NQ)
    g.nc = nc

    cs, rs = meta["cs"], meta["rs"]
    maxtg = 0
    for d in (cs, rs):
        for grp in d["groups"]:
            for h in (0, 1):
                maxtg = max(maxtg, sum(d["T"][b][h] for b in grp))
    g.maxtg = int(maxtg)

    # ---- params
    def par(name, shape, dt=F32):
        return nc.declare_dram_parameter(name, list(shape), dt, isOutput=False)

    xs0p = par("xs0", [Np, cfg.D])
    g.dirs = {}
    for nm, d in (("cs", cs), ("rs", rs)):
        g.dirs[nm] = dict(d)
        g.dirs[nm]["idx16_d"] = par(f"idx16_{nm}", [128, d["E_flat"] // 16], I16)
        g.dirs[nm]["dstl_d"] = par(f"dstl_{nm}", [128, d["ntiles"]])
    statc = par("statc", [P, g.bpc, 6])     # u0,q0,r0,aggr0,realmask_sh,sigma0
    rmfp = par("realmask_full", [P, g.BT])
    iotap = par("iotabig", [P, g.maxtg, P])
    identp = par("ident", [P, P])
    onesp = par("ones", [P, 2])             # col of ones; col 1 unused
    pbcp = par("pbc", [2 * P, P])           # p0,p1 broadcast tiles
    wallp = par("wall", [4 * P, P])         # f32 weights: P1,P2,P4,P5
    ballp = par("ball", [4 * P, P])          # bias rows replicated to 128 parts
    wallbp = par("wallb", [6 * P, P], BF16)  # bf16 weights: P7..P14
    ballbp = par("ballb", [6 * P, P], BF16)
    out_p = nc.declare_dram_parameter("out", [npc, cfg.D], F32, isOutput=True)

    # ---- internal dram
    def dram(name, shape, dt=F32, shared=False):
        return nc.dram_tensor(name, list(shape), dt,
                              addr_space="Shared" if shared else "Local")

    xs_w = {k: (192 if k == 3 else 256 if k == 6 else cfg.D)
            for k in range(1, 14)}
    xs_dt = {k: (F32 if k <= 5 else BF16) for k in range(1, 14)}
    XS = {k: dram(f"xs{k}", [Np, xs_w[k]], xs_dt[k], shared=True)
          for k in range(1, 14)}
    xsout = {k: dram(f"xso{k}", [npc, xs_w[k]], xs_dt[k]) for k in range(1, 14)}
    g.z_dram = dram("ztab", [Np, ZW])
    h2save = dram("h2save", [npc, cfg.D])
    h5save = dram("h5save", [npc, cfg.D])
    h3tmp = dram("h3tmp", [npc, cfg.D])
    h6tmp = dram("h6tmp", [npc, cfg.D])
    score_sh_d = {i: dram(f"scsh{i}", [P, g.bpc]) for i in (0, 1)}
    score_fl_d = {i: dram(f"scfl{i}", [NCORES * P, g.bpc], shared=True)
                  for i in (0, 1)}

    with tile.TileContext(nc) as tc:
        g.tc = tc
        ctxs = [
            tc.tile_pool(name="const", bufs=1),
            tc.tile_pool(name="nvp", bufs=1),
            tc.tile_pool(name="idxp", bufs=4),
            tc.tile_pool(name="gathp", bufs=4),
            tc.tile_pool(name="ohtp", bufs=4),
            tc.tile_pool(name="evp", bufs=3),
            tc.tile_pool(name="outp", bufs=3),
            tc.tile_pool(name="zstp", bufs=1),
            tc.tile_pool(name="bisp", bufs=1),
            tc.tile_pool(name="psagg", bufs=4, space="PSUM"),
            tc.tile_pool(name="psw", bufs=2, space="PSUM"),
            tc.tile_pool(name="pst", bufs=2, space="PSUM"),
        ]
        cpool, g.sb_nv, g.sb_idx, g.sb_gath, g.sb_oht, g.sb_ev, g.sb_out, \
            g.sb_zst, g.sb_bis, g.ps_agg, g.ps_w, g.ps_t = \
            [c.__enter__() for c in ctxs]

        # ---- constants into sbuf
        def cload(ap_src, shape, tag, dt=F32):
            t = cpool.tile(list(shape), dt, name=tag, tag=tag)
            nc.sync.dma_start(out=t[:], in_=ap_src)
            return t

        g.iota_big = cload(iotap.ap(), [P, g.maxtg, P], "iota")
        g.ident = cload(identp.ap(), [P, P], "ident")
        g.ident_bf = cpool.tile([P, P], BF16, name="identb", tag="identb")
        nc.vector.tensor_copy(out=g.ident_bf[:], in_=g.ident[:])
        ones2 = cload(onesp.ap(), [P, 2], "ones2")
        g.ones_col = ones2[:, 0:1]
        orow = cpool.tile([1, P], F32, name="orow", tag="orow")
        nc.vector.memset(orow[:], 1.0)
        g.ones_row = orow
        statc_t = cload(statc.ap(), [P, g.bpc, 6], "statc")
        u0c = statc_t[:, :, 0]
        q0c = statc_t[:, :, 1]
        r0c = statc_t[:, :, 2]
        aggr0c = statc_t[:, :, 3]
        rm_sh = statc_t[:, :, 4]
        sig0col = statc_t[:, :, 5]
        rm_fl = cload(rmfp.ap(), [P, g.BT], "rmfl")
        pbc_t = cload(pbcp.ap().rearrange("(v p) d -> p v d", p=P), [P, 2, P], "pbc")
        p0bc, p1bc = pbc_t[:, 0, :], pbc_t[:, 1, :]
        wall = cload(wallp.ap().rearrange("(w p) d -> p w d", p=P),
                     [P, 4, P], "wall")
        ball = cload(ballp.ap().rearrange("(w p) d -> p w d", p=P),
                     [P, 4, P], "ball")
        wallb = cload(wallbp.ap().rearrange("(w p) d -> p w d", p=P),
                      [P, 6, P], "wallb", BF16)
        ballb = cload(ballbp.ap().rearrange("(w p) d -> p w d", p=P),
                      [P, 6, P], "ballb", BF16)
        Wt = [wall[:, i, :] for i in range(4)]
        bt = [ball[:, i, :] for i in range(4)]
        Wtb = [wallb[:, i, :] for i in range(6)]
        btb = [ballb[:, i, :] for i in range(6)]

        # one-time scrub of the gather slots so skipped (trailing-negative)
        # slots never feed NaN garbage into matmuls; shape must be the
        # byte-largest user of the tag (f32 elem=192)
        for _ in range(4):
            z = g.sb_gath.tile([P, g.maxtg, 192], F32, name="fg", tag="fg")
            nc.vector.memset(z[:], 0.0)

        # persistent z-derived columns (filled at P4/P7 evictions)
        sig1col = cpool.tile([P, g.bpc], F32, name="sig1c", tag="sig1c")
        sig2col = cpool.tile([P, g.bpc], F32, name="sig2c", tag="sig2c")
        A1col = cpool.tile([P, g.bpc], F32, name="A1c", tag="A1c")

        def vmul(a, b_, tag):
            t = nv(g, tag)
            nc.vector.tensor_tensor(out=t[:], in0=a[:], in1=b_[:], op=OP.mult)
            return t

        u0u0 = vmul(u0c, u0c, "u0u0")
        u0q0 = vmul(u0c, q0c, "u0q0")

        def scoped(name):
            return nc.named_scope(name)

        def emit_schedule():
            # =========== DOWN LEVEL 0 ===========
            with scoped("P1"):
                emit_dir_pass(g, "cs", xs0p, "conv",
                              conv_block_fn(g, Wt[0], bt[0],
                                            [("xs", xsout[1], u0u0)],
                                            sigma_col=sig0col))
                allgather(g, xsout[1], XS[1])
            with scoped("P2"):
                emit_dir_pass(g, "cs", XS[1], "conv",
                              conv_block_fn(g, Wt[1], bt[1],
                                            [("xs", xsout[2], u0q0),
                                             ("xs", h2save, u0c)],
                                            sigma_col=sig0col))
                allgather(g, xsout[2], XS[2])
            with scoped("P3"):
                score0 = nv(g, "score0")
                emit_dir_pass(g, "cs", XS[2], "wec",
                              wec_block_fn(g, r0c, [("xs", h3tmp, r0c)],
                                           score_to=score0, pbc=p0bc))
                nc.sync.dma_start(out=score_sh_d[0].ap(), in_=score0[:])
                allgather(g, score_sh_d[0], score_fl_d[0])
            with scoped("topk0"):
                sc0f = load_full_from_ag(g, score_fl_d[0], "sc0f")
                sel0f = sel_from(g, sc0f, rm_fl, "sel0f")
                k0 = math.ceil(cfg.ratio * cfg.N)
                thr0 = bisect_topk(g, sel0f, k0, "thr0")
                kept0f = nv(g, "kept0f", [P, g.BT])
                nc.vector.tensor_scalar(out=kept0f[:], in0=sel0f[:],
                                        scalar1=thr0[:], scalar2=None,
                                        op0=OP.is_ge)
                sel0s = sel_from(g, score0, rm_sh, "sel0s")
                kept0s = nv(g, "kept0s")
                nc.vector.tensor_scalar(out=kept0s[:], in0=sel0s[:],
                                        scalar1=thr0[:], scalar2=None,
                                        op0=OP.is_ge)
                tanh0 = nv(g, "tanh0")
                nc.scalar.activation(out=tanh0[:], in_=score0[:], func=AF.Tanh)
            # Z pass A: deg1 raw (rs direction, gather kept0 at col, segsum by row)
            with scoped("ZA"):
                zbuild(g, [kept0f])
                S1 = nv(g, "S1")
                emit_dir_pass(g, "rs", None, "z",
                              z_block_fn(g, [], [(S1, 0)], 1), zcols=1)
            with scoped("lvl1fac"):
                deg1 = vmul(kept0s, S1, "deg1")
                m1 = nv(g, "m1")
                nc.vector.tensor_scalar(out=m1[:], in0=deg1[:], scalar1=0.0,
                                        scalar2=None, op0=OP.is_gt)
                dsafe = nv(g, "dsafe")
                nc.vector.tensor_scalar(out=dsafe[:], in0=deg1[:], scalar1=1e-30,
                                        scalar2=None, op0=OP.max)
                u1 = nv(g, "u1")
                nc.vector.reciprocal(out=u1[:], in_=dsafe[:])
                nc.scalar.activation(out=u1[:], in_=u1[:], func=AF.Sqrt)
                nc.vector.tensor_tensor(out=u1[:], in0=u1[:], in1=m1[:], op=OP.mult)
                w1 = vmul(aggr0c, kept0s, "w1")
                rdeg1 = nv(g, "rdeg1")
                nc.vector.reciprocal(out=rdeg1[:], in_=dsafe[:])
                q1 = vmul(w1, rdeg1, "q1")
                nc.vector.tensor_tensor(out=q1[:], in0=q1[:], in1=m1[:], op=OP.mult)
                # XS3 = [h3 * tanh0 * u1 | u1 | q1 | pad]
                cv3 = vmul(tanh0, u1, "cv3")
                for b in range(g.bpc):
                    t = g.sb_out.tile([P, P], F32, name="rs3", tag="rs3")
                    nc.sync.dma_start(
                        out=t[:],
                        in_=h3tmp.ap().rearrange("(b p) d -> b p d", p=P)[b])
                    nc.vector.tensor_scalar(out=t[:], in0=t[:],
                                            scalar1=cv3[:, b : b + 1],
                                            scalar2=None, op0=OP.mult)
                    nc.sync.dma_start(
                        out=xsout[3].ap().rearrange("(b p) d -> b p d", p=P)
                        [b][:, 0:P], in_=t[:])
                uq = g.sb_out.tile([P, g.bpc, 2], F32, name="uq", tag="uq")
                nc.vector.tensor_copy(out=uq[:, :, 0:1], in_=u1[:, :, None])
                nc.vector.tensor_copy(out=uq[:, :, 1:2], in_=q1[:, :, None])
                nc.sync.dma_start(
                    out=xsout[3].ap().rearrange("(b p) d -> p b d", p=P)
                    [:, :, P : P + 2], in_=uq[:])
                allgather(g, xsout[3], XS[3])

            # =========== DOWN LEVEL 1 ===========
            u1u1 = vmul(u1, u1, "u1u1")
            u1q1 = vmul(u1, q1, "u1q1")
            with scoped("P4"):
                emit_dir_pass(
                    g, "cs", XS[3], "conv",
                    conv_block_fn(g, Wt[2], bt[2], [("xs", xsout[4], u1u1)],
                                  zinfo={"zc": 2,
                                         "cols": [(sig1col, 0), (A1col, 1)],
                                         "bias_col": (sig1col, 0)}),
                    elem=192, zmm=2)
                allgather(g, xsout[4], XS[4])
            with scoped("lvl1fac2"):
                aggr1 = vmul(kept0s, A1col, "aggr1")
                nc.vector.tensor_scalar(out=aggr1[:], in0=aggr1[:], scalar1=1e-12,
                                        scalar2=None, op0=OP.add)
                raggr1 = nv(g, "raggr1")
                nc.vector.reciprocal(out=raggr1[:], in_=aggr1[:])
                r1 = vmul(kept0s, raggr1, "r1")
            with scoped("P5"):
                emit_dir_pass(g, "cs", XS[4], "conv",
                              conv_block_fn(g, Wt[3], bt[3],
                                            [("xs", xsout[5], u1q1),
                                             ("xs", h5save, u1)],
                                            sigma_col=sig1col))
                allgather(g, xsout[5], XS[5])
            with scoped("P6"):
                score1 = nv(g, "score1")
                emit_dir_pass(g, "cs", XS[5], "wec",
                              wec_block_fn(g, r1, [("xs", h6tmp, r1)],
                                           score_to=score1, pbc=p1bc))
                nc.sync.dma_start(out=score_sh_d[1].ap(), in_=score1[:])
                allgather(g, score_sh_d[1], score_fl_d[1])
            with scoped("topk1"):
                sc1f = load_full_from_ag(g, score_fl_d[1], "sc1f")
                sel1f = sel_from(g, sc1f, kept0f, "sel1f")
                k0 = math.ceil(cfg.ratio * cfg.N)
                k1 = math.ceil(cfg.ratio * k0)
                thr1 = bisect_topk(g, sel1f, k1, "thr1")
                kept1f = nv(g, "kept1f", [P, g.BT])
                nc.vector.tensor_scalar(out=kept1f[:], in0=sel1f[:],
                                        scalar1=thr1[:], scalar2=None,
                                        op0=OP.is_ge)
                sel1s = sel_from(g, score1, kept0s, "sel1s")
                kept1s = nv(g, "kept1s")
                nc.vector.tensor_scalar(out=kept1s[:], in0=sel1s[:],
                                        scalar1=thr1[:], scalar2=None,
                                        op0=OP.is_ge)
                tanh1 = nv(g, "tanh1")
                nc.scalar.activation(out=tanh1[:], in_=score1[:], func=AF.Tanh)
            # Z pass C: deg2 raw
            with scoped("ZC"):
                zbuild(g, [kept1f])
                S2 = nv(g, "S2")
                emit_dir_pass(g, "rs", None, "z",
                              z_block_fn(g, [], [(S2, 0)], 1), zcols=1)
            with scoped("lvl2fac"):
                deg2 = vmul(kept1s, S2, "deg2")
                m2 = nv(g, "m2")
                nc.vector.tensor_scalar(out=m2[:], in0=deg2[:], scalar1=0.0,
                                        scalar2=None, op0=OP.is_gt)
                d2safe = nv(g, "d2safe")
                nc.vector.tensor_scalar(out=d2safe[:], in0=deg2[:], scalar1=1e-30,
                                        scalar2=None, op0=OP.max)
                u2 = nv(g, "u2")
                nc.vector.reciprocal(out=u2[:], in_=d2safe[:])
                nc.scalar.activation(out=u2[:], in_=u2[:], func=AF.Sqrt)
                nc.vector.tensor_tensor(out=u2[:], in0=u2[:], in1=m2[:], op=OP.mult)
                # XS6 (bf16) = [h6 * tanh1 * u2 | u2 | pad]
                cv6 = vmul(tanh1, u2, "cv6")
                for b in range(g.bpc):
                    t = g.sb_out.tile([P, P], F32, name="rs6", tag="rs6")
                    nc.sync.dma_start(
                        out=t[:],
                        in_=h6tmp.ap().rearrange("(b p) d -> b p d", p=P)[b])
                    tb = g.sb_out.tile([P, P], BF16, name="rs6b", tag="rs6b")
                    nc.vector.tensor_scalar(out=tb[:], in0=t[:],
                                            scalar1=cv6[:, b : b + 1],
                                            scalar2=None, op0=OP.mult)
                    nc.sync.dma_start(
                        out=xsout[6].ap().rearrange("(b p) d -> b p d", p=P)
                        [b][:, 0:P], in_=tb[:])
                u2b = g.sb_out.tile([P, g.bpc, 1], BF16, name="u2b", tag="u2b")
                nc.vector.tensor_copy(out=u2b[:, :, 0:1], in_=u2[:, :, None])
                nc.sync.dma_start(
                    out=xsout[6].ap().rearrange("(b p) d -> p b d", p=P)
                    [:, :, P : P + 1], in_=u2b[:])
                allgather(g, xsout[6], XS[6])

            # =========== BOTTOM ===========
            u2u2 = vmul(u2, u2, "u2u2")
            u2r1 = vmul(u2, r1, "u2r1")
            with scoped("P7"):
                emit_dir_pass(
                    g, "cs", XS[6], "conv",
                    conv_block_fn(g, Wtb[0], btb[0], [("xs", xsout[7], u2u2)],
                                  dt=BF16,
                                  zinfo={"zc": 1,
                                         "cols": [(sig2col, 0)],
                                         "bias_col": (sig2col, 0)}),
                    elem=256, dt=BF16, zmm=1)
                allgather(g, xsout[7], XS[7])
            with scoped("P8"):
                emit_dir_pass(g, "cs", XS[7], "conv",
                              conv_block_fn(g, Wtb[1], btb[1],
                                            [("xs", xsout[8], u2r1)],
                                            dt=BF16, sigma_col=sig2col),
                              dt=BF16)
                allgather(g, xsout[8], XS[8])

            # =========== UP LEVEL (uses emask1): wec-up + 2 convs ===========
            q1u1 = vmul(q1, u1, "q1u1")
            with scoped("P9"):
                emit_dir_pass(g, "rs", XS[8], "wec",
                              wec_block_fn(g, q1, [("xs", xsout[9], q1u1)]),
                              dt=BF16)
                allgather(g, xsout[9], XS[9])
            with scoped("P10"):
                emit_dir_pass(g, "cs", XS[9], "conv",
                              conv_block_fn(g, Wtb[2], btb[2],
                                            [("xs", xsout[10], u1u1)],
                                            dt=BF16, sigma_col=sig1col),
                              dt=BF16)
                allgather(g, xsout[10], XS[10])
            with scoped("P11"):
                emit_dir_pass(g, "cs", XS[10], "conv",
                              conv_block_fn(g, Wtb[3], btb[3],
                                            [("addshard", h5save, xsout[11],
                                              u1, r0c)],
                                            dt=BF16, sigma_col=sig1col),
                              dt=BF16)
                allgather(g, xsout[11], XS[11])

            # =========== UP LEVEL (emask0) ===========
            q0u0 = vmul(q0c, u0c, "q0u0")
            with scoped("P12"):
                emit_dir_pass(g, "rs", XS[11], "wec",
                              wec_block_fn(g, q0c, [("xs", xsout[12], q0u0)]),
                              dt=BF16)
                allgather(g, xsout[12], XS[12])
            with scoped("P13"):
                emit_dir_pass(g, "cs", XS[12], "conv",
                              conv_block_fn(g, Wtb[4], btb[4],
                                            [("xs", xsout[13], u0u0)],
                                            dt=BF16, sigma_col=sig0col),
                              dt=BF16)
                allgather(g, xsout[13], XS[13])
            with scoped("P14"):
                emit_dir_pass(g, "cs", XS[13], "conv",
                              conv_block_fn(g, Wtb[5], btb[5],
                                            [("addshard", h2save, out_p,
                                              u0c, None)],
                                            dt=BF16, sigma_col=sig0col),
                              dt=BF16)

        emit_schedule()

        for c in reversed(ctxs):
            c.__exit__(None, None, None)

    nc.compile()
    return nc


def make_inmaps(cfg, meta):
    cs, rs = meta["cs"], meta["rs"]
    bpc = cfg.blocks_per_core
    npc = cfg.nodes_per_core
    sv = meta["shardvec"]
    u0s, q0s, r0s, ag0s = (sv(meta[k]) for k in ("u0", "q0", "r0", "aggr0"))
    rms = sv(meta["realmask"])
    sig0s = sv(meta["sigma0"])
    rmf = meta["fullvec"](meta["realmask"])
    maxtg = 0
    for d in (cs, rs):
        for grp in d["groups"]:
            for h in (0, 1):
                maxtg = max(maxtg, sum(d["T"][b][h] for b in grp))
    iota = np.tile(np.arange(P, dtype=np.float32)[None, None, :], (P, maxtg, 1))
    ident = np.eye(P, dtype=np.float32)
    ones = np.ones((P, 2), np.float32)
    pbc = np.concatenate([
        np.tile(meta["p0"][None, :], (P, 1)),
        np.tile(meta["p1"][None, :], (P, 1))], 0).astype(np.float32)

    in_maps = []
    for c in range(NCORES):
        statcv = np.zeros((P, bpc, 6), np.float32)
        statcv[:, :, 0] = u0s[c]
        statcv[:, :, 1] = q0s[c]
        statcv[:, :, 2] = r0s[c]
        statcv[:, :, 3] = ag0s[c]
        statcv[:, :, 4] = rms[c]
        statcv[:, :, 5] = sig0s[c]
        in_maps.append({
            "xs0": meta["xs0"],
            "idx16_cs": cs["idx16"][c], "dstl_cs": cs["dstl"][c],
            "idx16_rs": rs["idx16"][c], "dstl_rs": rs["dstl"][c],
            "statc": statcv,
            "realmask_full": rmf,
            "iotabig": iota, "ident": ident, "ones": ones, "pbc": pbc,
            "wall": None, "ball": None, "wallb": None, "ballb": None,
        })
    return in_maps


def fill_weights(in_maps, Wd, bd, Wu, bu, Wb, bb):
    Wf = [Wd[0, 0], Wd[0, 1], Wd[1, 0], Wd[1, 1]]
    bf = [bd[0, 0], bd[0, 1], bd[1, 0], bd[1, 1]]
    Wb16 = [Wb[0], Wb[1], Wu[0, 0], Wu[0, 1], Wu[1, 0], Wu[1, 1]]
    bb16 = [bb[0], bb[1], bu[0, 0], bu[0, 1], bu[1, 0], bu[1, 1]]
    wall = np.concatenate([w.astype(np.float32) for w in Wf], 0)
    ball = np.concatenate([np.tile(b.astype(np.float32)[None, :], (P, 1))
                           for b in bf], 0)
    wallb = np.concatenate([w for w in Wb16], 0).astype(ml_dtypes.bfloat16)
    ballb = np.concatenate([np.tile(np.asarray(b)[None, :], (P, 1))
                            for b in bb16], 0).astype(ml_dtypes.bfloat16)
    for m in in_maps:
        m["wall"] = wall
        m["ball"] = ball
        m["wallb"] = wallb
        m["ballb"] = ballb


def run_gnn(cfg, inputs, nc_cache={}, full_pad=False, trace=False):
    """Full pipeline: preprocess, build (cached by cfg), run, assemble."""
    x = np.asarray(inputs["x"], np.float32)
    ei = np.asarray(inputs["edge_index"])
    pvec = np.asarray(inputs["pvec"], np.float32)
    meta = preprocess(cfg, x, ei, pvec)
    key = (cfg.N, cfg.E, ei.tobytes()[:64])  # program depends on edge stats
    if key not in nc_cache:
        nc_cache.clear()
        nc_cache[key] = (build_kernel(cfg, meta), None)
    nc, _ = nc_cache[key]
    in_maps = make_inmaps(cfg, meta)
    fill_weights(in_maps, *(np.asarray(inputs[k], np.float32)
                            for k in ("Wd", "bd", "Wu", "bu", "Wb", "bb")))
    res = run_bass_kernel_spmd(nc, in_maps, list(range(NCORES)), trace=trace)
    out = np.concatenate([res.results[c]["out"] for c in range(NCORES)], 0)
    return (out if full_pad else out[: cfg.N]), res


_CFG = Cfg()


def kernel(**inputs):
    out, _ = run_gnn(_CFG, inputs)
    return out.astype(np.float32)


# revision 15
# speedup vs baseline: 1.5294x; 1.0206x over previous
"""Trainium2 Bass kernel for nn_MessagePassingLayer (graph U-Net message
passing) on 8 NeuronCores.

Self-contained: kernel(**inputs) takes the full unsharded inputs and
returns the full [50000, 128] float32 output.

Strategy: nodes padded to 50176 and sharded contiguously over the 8
cores; edges bucketed per (dst block, src half) in destination-sorted
order. Every per-edge coefficient in this network factorizes as
rowfac[row[e]] * colfac[col[e]], so row factors are pre-applied to the
gathered node table (per-pass XS buffers, exchanged via AllGather) and
col factors are applied per-partition on PSUM eviction.

Perf structure (v2):
 - dma_gather calls round-robin over 4 SWDGE queues (desc gen runs on
   distinct Q7 core pairs in parallel).
 - passes P7..P14 (bottom + up path, after both top-k selections) run
   with bf16 gather tables and bf16 matmuls; P1..P6 stay f32 so the
   top-k thresholds match the reference bit-for-bit.
 - the sigma1/A1 and sigma2 z-passes are folded into P4/P7 as extra
   gathered columns + a second per-tile matmul into a spare PSUM
   region; only the two deg z-passes (rs direction) remain.
 - trailing padding slots of each gather call use idx=-1 (descriptor
   emission skipped by the Q7 ucode).
"""
import math
import os

import numpy as np
import ml_dtypes
import concourse.bacc as bacc
import concourse.mybir as mybir
import concourse.tile as tile
from concourse.bass_utils import run_bass_kernel_spmd

from dataclasses import dataclass, field

P = 128
NCORES = 8
NQ = 4            # SWDGE queues used round-robin for gathers


@dataclass
class Cfg:
    N: int = 50000
    E: int = 800000
    D: int = 128
    L: int = 2
    B: int = 2
    ratio: float = 0.5
    # derived
    blocks_per_core: int = field(init=False)
    N_pad: int = field(init=False)
    nodes_per_core: int = field(init=False)
    half: int = field(init=False)          # rows per gather half-table

    def __post_init__(self):
        blocks_total = math.ceil(self.N / P)
        self.blocks_per_core = math.ceil(blocks_total / NCORES)
        self.N_pad = self.blocks_per_core * NCORES * P
        self.nodes_per_core = self.blocks_per_core * P
        self.half = self.N_pad // 2
        assert self.half <= 32768, "gather half-table must fit int16 index"
        assert self.N_pad // 2 % P == 0


def wrap_idx(idx, n):
    """[n] int -> [128, n/16] int16 wrapped+replicated layout for dma_gather."""
    assert n % 16 == 0
    w = np.zeros((16, n // 16), np.int16)
    w[np.arange(n) % 16, np.arange(n) // 16] = idx.astype(np.int16)
    return np.tile(w, (8, 1))


def build_direction(cfg, src, dst, group_blocks):
    """Static tables for one scatter direction.

    src/dst: [E] global endpoint arrays (gather at src, scatter to dst).
    Edges are bucketed per (core, local dst block, src half), padded to
    tiles of 128 with null edges. Tile counts per (block position, half)
    are maxed across cores so the 8 cores share one instruction stream.
    Null slots that end up at the tail of a gather call get idx=-1 (the
    gather ucode skips trailing negative indices); interior nulls gather
    the always-zero pad row.
    """
    bpc = cfg.blocks_per_core
    npc = cfg.nodes_per_core
    half = cfg.half
    zero_row = cfg.N_pad - 1          # a pad node: always-zero row (half 1)

    buckets = [[[None, None] for _ in range(bpc)] for _ in range(NCORES)]
    core_of = dst // npc
    blk_of = (dst % npc) // P
    half_of = (src >= half).astype(np.int64)
    order = np.lexsort((src, half_of, blk_of, core_of))
    key = ((core_of[order] * bpc) + blk_of[order]) * 2 + half_of[order]
    bounds = np.searchsorted(key, np.arange(NCORES * bpc * 2 + 1))
    for c in range(NCORES):
        for b in range(bpc):
            for h in (0, 1):
                kk = (c * bpc + b) * 2 + h
                s, e = bounds[kk], bounds[kk + 1]
                buckets[c][b][h] = order[s:e]

    T = np.zeros((bpc, 2), np.int64)
    for b in range(bpc):
        for h in (0, 1):
            mx = max(len(buckets[c][b][h]) for c in range(NCORES))
            T[b, h] = max(1, math.ceil(mx / P))

    groups = []
    for g0 in range(0, bpc, group_blocks):
        groups.append(list(range(g0, min(g0 + group_blocks, bpc))))

    tot_tiles = int(T.sum())
    E_flat = tot_tiles * P
    idxs = np.zeros((NCORES, E_flat), np.int64)
    dstl = np.zeros((NCORES, E_flat), np.float32)
    pos = 0
    tile_plan = []   # (group, h, b, ntiles, start_pos) shared across cores
    for grp in groups:
        for h in (0, 1):
            for gi, b in enumerate(grp):
                nt = int(T[b, h])
                last_in_call = gi == len(grp) - 1
                tile_plan.append((h, b, nt, pos))
                for c in range(NCORES):
                    ed = buckets[c][b][h]
                    n = len(ed)
                    assert n <= nt * P
                    sl = slice(pos, pos + n)
                    idxs[c, sl] = src[ed] - h * half
                    dstl[c, sl] = (dst[ed] % npc) % P
                    if n < nt * P:
                        psl = slice(pos + n, pos + nt * P)
                        dstl[c, psl] = 200.0       # never matches iota
                        idxs[c, psl] = (zero_row - half) if h == 1 else 0
                pos += nt * P
    assert pos == E_flat

    idx16 = np.stack([wrap_idx(idxs[c], E_flat) for c in range(NCORES)])
    ntiles = E_flat // P
    dstl_t = dstl.reshape(NCORES, ntiles, P).transpose(0, 2, 1).copy()

    return {
        "idx16": idx16,            # [NCORES, 128, E_flat/16] int16
        "dstl": dstl_t,            # [NCORES, 128, ntiles] f32
        "tile_plan": tile_plan,    # shared: (h, b, ntiles, start_pos)
        "groups": groups,
        "E_flat": E_flat,
        "ntiles": ntiles,
        "T": T,
    }


def preprocess(cfg, x, edge_index, pvec):
    """All static host work. Returns per-core input maps pieces + meta."""
    N, Np = cfg.N, cfg.N_pad
    row = edge_index[0].astype(np.int64)
    col = edge_index[1].astype(np.int64)

    deg0 = np.zeros(Np, np.float32)
    np.add.at(deg0, row, 1.0)
    with np.errstate(divide="ignore"):
        dis0 = np.where(deg0 > 0, deg0.astype(np.float64) ** -0.5, 0.0
                        ).astype(np.float32)
        normed0 = np.where(deg0 > 0, 1.0 / np.where(deg0 > 0, deg0, 1), 0.0
                           ).astype(np.float32)
    A0 = np.zeros(Np, np.float32)
    np.add.at(A0, col, normed0[row])
    aggr0 = (A0 + 1e-12).astype(np.float32)
    r0 = (1.0 / aggr0).astype(np.float32)
    q0 = normed0
    u0 = dis0
    sigma0 = np.zeros(Np, np.float32)
    np.add.at(sigma0, col, u0[row])

    cs = build_direction(cfg, row, col, group_blocks=2)
    rs = build_direction(cfg, col, row, group_blocks=2)

    xs0 = np.zeros((Np, cfg.D), np.float32)
    xs0[:N] = x * u0[:N, None]

    realmask = np.zeros(Np, np.float32)
    realmask[:N] = 1.0

    p0 = pvec[0] / np.linalg.norm(pvec[0])
    p1 = pvec[1] / np.linalg.norm(pvec[1])

    def shardvec(v):   # [Np] -> [NCORES, 128, bpc] (partition-major per block)
        return v.reshape(NCORES, cfg.blocks_per_core, P).transpose(0, 2, 1).copy()

    def fullvec(v):    # [Np] -> [128, blocks_total]
        return v.reshape(-1, P).T.copy()

    meta = {
        "cs": cs, "rs": rs,
        "u0": u0, "q0": q0, "r0": r0, "sigma0": sigma0, "aggr0": aggr0,
        "xs0": xs0, "realmask": realmask, "p0": p0, "p1": p1,
        "shardvec": shardvec, "fullvec": fullvec,
    }
    return meta


F32 = mybir.dt.float32
BF16 = mybir.dt.bfloat16
I16 = mybir.dt.int16
AF = mybir.ActivationFunctionType
OP = mybir.AluOpType
AX = mybir.AxisListType

ZCHUNK = 40
ZW = 64
BISECT_ITERS = 34
BISECT_RANGE = 1024.0    # |score| << 512; threshold is within hi-1024..hi


class G:
    """build-time globals bag"""
    pass


# ------------------------------------------------------------- edge passes --

def emit_dir_pass(g, dirn, src_dram, mode, block_fn, zcols=0,
                  elem=128, dt=F32, zmm=0):
    """One edge pass.

    mode: 'conv' (psum [f,d]), 'wec' (psum [d,f]), 'z' (psum [zcols,d]).
    elem: gathered row width in dt elements; cols [P, P+zmm) are per-src
    scalars accumulated into psum region [0:zmm, P:2P] (conv mode only).
    """
    nc = g.nc
    d = g.dirs[dirn]
    if mode == "z":
        elem = ZW
        in_aps = [g.z_dram.ap()[0 : g.half, :], g.z_dram.ap()[g.half :, :]]
    else:
        in_aps = [src_dram.ap()[0 : g.half, :], src_dram.ap()[g.half :, :]]

    plan = {(h, b): (nt, pos) for (h, b, nt, pos) in d["tile_plan"]}
    psums = {}
    done = {}
    for grp in d["groups"]:
        parts = {}
        for h in (0, 1):
            t0 = plan[(h, grp[0])][1] // P
            ntg = sum(plan[(h, b)][0] for b in grp)
            nidx = ntg * P
            it = g.sb_idx.tile([128, nidx // 16], I16, name="idx", tag="idx")
            nc.sync.dma_start(
                out=it[:], in_=d["idx16_d"].ap()[:, t0 * 8 : t0 * 8 + nidx // 16])
            gt = g.sb_gath.tile([P, ntg, elem], dt, name="fg", tag="fg")
            nc.gpsimd.dma_gather(
                out_ap=gt[:], in_ap=in_aps[h], idxs_ap=it[:],
                num_idxs=nidx, num_idxs_reg=nidx, elem_size=elem,
                single_packet=False, queue_num=g.qctr % NQ)
            g.qctr += 1
            dl = g.sb_idx.tile([P, ntg], F32, name="dl", tag="dl")
            nc.sync.dma_start(out=dl[:], in_=d["dstl_d"].ap()[:, t0 : t0 + ntg])
            oh = g.sb_oht.tile([P, ntg, P], dt, name="oht", tag="oht")
            nc.vector.tensor_tensor(
                out=oh[:], in0=dl[:, :, None].to_broadcast([P, ntg, P]),
                in1=g.iota_big[:, : ntg, :], op=OP.is_equal)
            parts[h] = (gt, oh, t0)
        for h in (0, 1):
            gt, oh, t0 = parts[h]
            for b in grp:
                nt, pos = plan[(h, b)]
                rel = pos // P - t0
                if b not in psums:
                    if mode == "z":
                        pp = [zcols, P]
                    elif zmm:
                        pp = [P, 2 * P]
                    else:
                        pp = [P, P]
                    psums[b] = g.ps_agg.tile(pp, F32, space="PSUM", name="agg",
                                             tag="agg")
                    done[b] = 0
                tot = plan[(0, b)][0] + plan[(1, b)][0]
                for t in range(nt):
                    done[b] += 1
                    first, last = done[b] == 1, done[b] == tot
                    if mode == "conv":
                        nc.tensor.matmul(out=psums[b][:, 0:P],
                                         lhsT=gt[:, rel + t, 0:P],
                                         rhs=oh[:, rel + t, :],
                                         start=first, stop=last)
                        if zmm:
                            nc.tensor.matmul(out=psums[b][0:zmm, P : 2 * P],
                                             lhsT=gt[:, rel + t, P : P + zmm],
                                             rhs=oh[:, rel + t, :],
                                             start=first, stop=last)
                    elif mode == "wec":
                        nc.tensor.matmul(out=psums[b][:, 0:P],
                                         lhsT=oh[:, rel + t, :],
                                         rhs=gt[:, rel + t, 0:P],
                                         start=first, stop=last)
                    else:
                        nc.tensor.matmul(out=psums[b][:],
                                         lhsT=gt[:, rel + t, :zcols],
                                         rhs=oh[:, rel + t, :],
                                         start=first, stop=last)
                    if last:
                        block_fn(b, psums[b])
                        del psums[b], done[b]


def conv_block_fn(g, W_sb, b_bc, outs, dt=F32, sigma_col=None, zinfo=None):
    """Per-block eviction for conv passes.

    Bias is applied on DVE at eviction: out += b_bc * (sigma[d] * scale[d]).
    sigma_col: [P, bpc] per-dst sigma shard (used when zinfo is None).
    zinfo: dict(zc=n, cols=[(col_tile, j), ...], bias_col=(tile, j)) --
    sigma/z data come from the pass's own PSUM z region [0:zc, P:2P],
    transposed per block into column tiles.
    """
    nc = g.nc
    ident = g.ident if dt == F32 else g.ident_bf

    def fn(b, pag):
        a1 = g.sb_ev.tile([P, P], dt, name="a1", tag="a1")
        if dt == BF16:
            nc.scalar.activation(out=a1[:], in_=pag[:, 0:P], func=AF.Copy)
        else:
            nc.vector.tensor_copy(out=a1[:], in_=pag[:, 0:P])
        if zinfo is not None:
            zc = zinfo["zc"]
            sigz = g.sb_ev.tile([2, P], dt, name="sigz", tag="sigz")
            nc.vector.tensor_copy(out=sigz[:zc, :], in_=pag[0:zc, P : 2 * P])
            pz = g.ps_t.tile([P, P], dt, space="PSUM", name="pst", tag="pst")
            nc.tensor.transpose(out=pz[:, 0:zc], in_=sigz[0:zc, :],
                                identity=ident[0:zc, 0:zc])
            for (ct, j) in zinfo["cols"]:
                nc.vector.tensor_copy(out=ct[:, b : b + 1], in_=pz[:, j : j + 1])
            bt, bj = zinfo["bias_col"]
            bias_col = bt[:, b : b + 1]
        else:
            bias_col = sigma_col[:, b : b + 1]
        p2 = g.ps_w.tile([P, P], F32, space="PSUM", name="p2", tag="p2")
        nc.tensor.matmul(out=p2[:], lhsT=W_sb[:], rhs=a1[:], start=True,
                         stop=True)
        a2 = g.sb_ev.tile([P, P], dt, name="a2", tag="a2")
        if dt == BF16:
            nc.scalar.activation(out=a2[:], in_=p2[:], func=AF.Copy)
        else:
            nc.vector.tensor_copy(out=a2[:], in_=p2[:])
        p3 = g.ps_t.tile([P, P], dt, space="PSUM", name="pst", tag="pst")
        nc.tensor.transpose(out=p3[:], in_=a2[:], identity=ident[:])
        emit_evictions(g, b, p3, outs, bias_bc=b_bc, bias_col=bias_col)
    return fn


def wec_block_fn(g, colfac, outs, score_to=None, pbc=None):
    nc = g.nc

    def fn(b, pag):
        if score_to is not None:
            sc = g.sb_ev.tile([P, P], F32, name="scm", tag="scm")
            nc.vector.tensor_tensor(out=sc[:], in0=pag[:, 0:P], in1=pbc[:],
                                    op=OP.mult)
            red = g.sb_ev.tile([P, 1], F32, name="scr", tag="scr")
            nc.vector.reduce_sum(red[:], sc[:], axis=AX.X)
            nc.vector.tensor_tensor(out=score_to[:, b : b + 1], in0=red[:],
                                    in1=colfac[:, b : b + 1], op=OP.mult)
        emit_evictions(g, b, pag, outs)
    return fn


def z_block_fn(g, row_to, col_to, zcols):
    """row_to: [(rowtile, j)] copy psum row j; col_to: [(coltile, j)]."""
    nc = g.nc

    def fn(b, pag):
        az = g.sb_ev.tile([P, P], F32, name="az", tag="az")
        nc.vector.tensor_copy(out=az[:zcols, :], in_=pag[:])
        for (rt, j) in row_to:
            nc.vector.tensor_copy(out=rt[:, b * P : (b + 1) * P],
                                  in_=az[j : j + 1, :])
        if col_to:
            pz = g.ps_t.tile([P, P], F32, space="PSUM", name="pst", tag="pst")
            nc.tensor.transpose(out=pz[:, :zcols], in_=az[:zcols, :],
                                identity=g.ident[:zcols, :zcols])
            for (ct, j) in col_to:
                nc.vector.tensor_copy(out=ct[:, b : b + 1],
                                      in_=pz[:, j : j + 1])
    return fn


def emit_evictions(g, b, psum, outs, bias_bc=None, bias_col=None):
    nc = g.nc
    for o in outs:
        kind = o[0]
        if kind == "xs":
            _, dram, scalevec = o
            dt_out = dram.dtype
            if bias_bc is not None:
                t = g.sb_out.tile([P, P], F32, name="xso", tag="xso")
                nc.scalar.activation(out=t[:], in_=psum[:, 0:P], func=AF.Copy,
                                     scale=scalevec[:, b : b + 1])
                to = g.sb_out.tile([P, P], dt_out, name="xso2", tag="xso2")
                sc2 = g.sb_out.tile([P, 1], F32, name="sc2", tag="sc2")
                nc.vector.tensor_tensor(out=sc2[:], in0=bias_col[:],
                                        in1=scalevec[:, b : b + 1], op=OP.mult)
                bt2 = g.sb_out.tile([P, P], F32, name="bt2", tag="bt2")
                nc.vector.tensor_scalar(out=bt2[:], in0=bias_bc[:],
                                        scalar1=sc2[:], scalar2=None,
                                        op0=OP.mult)
                nc.vector.tensor_tensor(out=to[:], in0=t[:], in1=bt2[:],
                                        op=OP.add)
            else:
                to = g.sb_out.tile([P, P], dt_out, name="xso2", tag="xso2")
                nc.scalar.activation(out=to[:], in_=psum[:, 0:P], func=AF.Copy,
                                     scale=scalevec[:, b : b + 1])
            nc.sync.dma_start(
                out=dram.ap().rearrange("(b p) d -> b p d", p=P)[b], in_=to[:])
        elif kind == "addshard":
            _, dram_in, dram_out, pre, post = o
            dt_out = dram_out.dtype
            t = g.sb_out.tile([P, P], F32, name="aso", tag="aso")
            nc.scalar.activation(out=t[:], in_=psum[:, 0:P], func=AF.Copy,
                                 scale=pre[:, b : b + 1])
            if bias_bc is not None:
                sc2 = g.sb_out.tile([P, 1], F32, name="sc2", tag="sc2")
                nc.vector.tensor_tensor(out=sc2[:], in0=bias_col[:],
                                        in1=pre[:, b : b + 1], op=OP.mult)
                bt2 = g.sb_out.tile([P, P], F32, name="bt2", tag="bt2")
                nc.vector.tensor_scalar(out=bt2[:], in0=bias_bc[:],
                                        scalar1=sc2[:], scalar2=None,
                                        op0=OP.mult)
                nc.vector.tensor_tensor(out=t[:], in0=t[:], in1=bt2[:],
                                        op=OP.add)
            sk = g.sb_out.tile([P, P], F32, name="skl", tag="skl")
            nc.sync.dma_start(
                out=sk[:], in_=dram_in.ap().rearrange("(b p) d -> b p d", p=P)[b])
            to = g.sb_out.tile([P, P], dt_out, name="aso2", tag="aso2")
            if post is not None:
                nc.vector.tensor_tensor(out=t[:], in0=t[:], in1=sk[:], op=OP.add)
                nc.vector.tensor_scalar(out=to[:], in0=t[:],
                                        scalar1=post[:, b : b + 1], scalar2=None,
                                        op0=OP.mult)
            else:
                nc.vector.tensor_tensor(out=to[:], in0=t[:], in1=sk[:], op=OP.add)
            nc.sync.dma_start(
                out=dram_out.ap().rearrange("(b p) d -> b p d", p=P)[b], in_=to[:])


# ------------------------------------------------------------- small pieces --

def allgather(g, in_dram, out_dram):
    g.nc.gpsimd.collective_compute(
        "AllGather", OP.bypass, replica_groups=[list(range(NCORES))],
        ins=[in_dram.ap()], outs=[out_dram.ap()])


def zbuild(g, cols):
    nc = g.nc
    for c0 in range(0, g.BT, ZCHUNK):
        nb = min(ZCHUNK, g.BT - c0)
        st = g.sb_zst.tile([P, ZCHUNK, ZW], F32, name="zst", tag="zst")
        for j, v in enumerate(cols):
            nc.vector.tensor_copy(out=st[:, :nb, j : j + 1],
                                  in_=v[:, c0 : c0 + nb, None])
        nc.sync.dma_start(
            out=g.z_dram.ap().rearrange("(b p) w -> p b w", p=P)[:, c0 : c0 + nb, :],
            in_=st[:, :nb, :])


def cross_part(g, col, op):
    nc = g.nc
    if op == "sum":
        pc = g.ps_t.tile([P, P], F32, space="PSUM", name="pst", tag="pst")
        nc.tensor.matmul(out=pc[:1, :1], lhsT=col[:], rhs=g.ones_col[:],
                         start=True, stop=True)
        out = g.sb_bis.tile([1, 1], F32, name="cnt", tag="cnt")
        nc.vector.tensor_copy(out=out[:], in_=pc[:1, :1])
        return out
    pt = g.ps_t.tile([P, P], F32, space="PSUM", name="pst", tag="pst")
    nc.tensor.transpose(out=pt[:1, :], in_=col[:], identity=g.ident[:])
    row = g.sb_bis.tile([1, P], F32, name="brow", tag="brow")
    nc.vector.tensor_copy(out=row[:], in_=pt[:1, :])
    out = g.sb_bis.tile([1, 1], F32, name="bred", tag="bred")
    nc.vector.reduce_max(out[:], row[:], axis=AX.X)
    return out


def bcast_scalar(g, s11, tag):
    nc = g.nc
    pb = g.ps_t.tile([P, P], F32, space="PSUM", name="pst", tag="pst")
    nc.tensor.matmul(out=pb[:, :1], lhsT=g.ones_row[:], rhs=s11[:],
                     start=True, stop=True)
    out = g.sb_nv.tile([P, 1], F32, name=tag, tag=tag)
    nc.vector.tensor_copy(out=out[:], in_=pb[:, :1])
    return out


def bisect_topk(g, sel_full, k, tag):
    """threshold col [128,1] such that count(sel >= t) == k exactly."""
    nc = g.nc
    mx = g.sb_bis.tile([P, 1], F32, name="bmx", tag="bmx")
    nc.vector.reduce_max(mx[:], sel_full[:], axis=AX.X)
    hi = cross_part(g, mx, "max")
    nc.vector.tensor_scalar(out=hi[:], in0=hi[:], scalar1=1.0, scalar2=None,
                            op0=OP.add)
    lo = g.sb_bis.tile([1, 1], F32, name="blo", tag="blo")
    nc.vector.tensor_scalar(out=lo[:], in0=hi[:], scalar1=-BISECT_RANGE,
                            scalar2=None, op0=OP.add)
    t = g.sb_bis.tile([1, 1], F32, name="bt", tag="bt")
    for _ in range(BISECT_ITERS):
        nc.vector.tensor_tensor(out=t[:], in0=lo[:], in1=hi[:], op=OP.add)
        nc.vector.tensor_scalar(out=t[:], in0=t[:], scalar1=0.5, scalar2=None,
                                op0=OP.mult)
        tcol = bcast_scalar(g, t, "btc")
        cmp = g.sb_bis.tile([P, g.BT], F32, name="bcmp", tag="bcmp")
        nc.vector.tensor_scalar(out=cmp[:], in0=sel_full[:], scalar1=tcol[:],
                                scalar2=None, op0=OP.is_ge)
        red = g.sb_bis.tile([P, 1], F32, name="bred2", tag="bred2")
        nc.vector.reduce_sum(red[:], cmp[:], axis=AX.X)
        cnt = cross_part(g, red, "sum")
        flag = g.sb_bis.tile([1, 1], F32, name="bflag", tag="bflag")
        nc.vector.tensor_scalar(out=flag[:], in0=cnt[:], scalar1=float(k) - 0.5,
                                scalar2=None, op0=OP.is_ge)
        d1 = g.sb_bis.tile([1, 1], F32, name="bd1", tag="bd1")
        nc.vector.tensor_tensor(out=d1[:], in0=t[:], in1=lo[:], op=OP.subtract)
        nc.vector.tensor_tensor(out=d1[:], in0=d1[:], in1=flag[:], op=OP.mult)
        nc.vector.tensor_tensor(out=lo[:], in0=lo[:], in1=d1[:], op=OP.add)
        nf = g.sb_bis.tile([1, 1], F32, name="bnf", tag="bnf")
        nc.vector.tensor_scalar(out=nf[:], in0=flag[:], scalar1=-1.0, scalar2=1.0,
                                op0=OP.mult, op1=OP.add)
        d2 = g.sb_bis.tile([1, 1], F32, name="bd2", tag="bd2")
        nc.vector.tensor_tensor(out=d2[:], in0=t[:], in1=hi[:], op=OP.subtract)
        nc.vector.tensor_tensor(out=d2[:], in0=d2[:], in1=nf[:], op=OP.mult)
        nc.vector.tensor_tensor(out=hi[:], in0=hi[:], in1=d2[:], op=OP.add)
    return bcast_scalar(g, lo, tag)


def load_full_from_ag(g, ag_dram, tag, nvec=1, vec=0):
    """AG out dram [(8*nvec*128), bpc] -> [128, BT] sbuf."""
    nc = g.nc
    out = g.sb_nv.tile([P, g.BT], F32, name=tag, tag=tag)
    for r in range(NCORES):
        src = ag_dram.ap().rearrange("(r v p) b -> r v p b", v=nvec, p=P)[r, vec]
        nc.sync.dma_start(out=out[:, r * g.bpc : (r + 1) * g.bpc], in_=src)
    return out


def nv(g, tag, shape=None):
    return g.sb_nv.tile(shape or [P, g.bpc], F32, name=tag, tag=tag)


def sel_from(g, score, active, tag):
    """sel = score*active + (active-1)*1e30 (elementwise, any width)."""
    nc = g.nc
    t1 = nv(g, tag, [P, score.shape[-1]])
    nc.vector.tensor_tensor(out=t1[:], in0=score[:], in1=active[:], op=OP.mult)
    t2 = nv(g, tag + "_m", [P, score.shape[-1]])
    nc.vector.tensor_scalar(out=t2[:], in0=active[:], scalar1=1e30,
                            scalar2=-1e30, op0=OP.mult, op1=OP.add)
    nc.vector.tensor_tensor(out=t1[:], in0=t1[:], in1=t2[:], op=OP.add)
    return t1


def build_kernel(cfg, meta):
    g = G()
    g.D = cfg.D
    g.half = cfg.half
    g.bpc = cfg.blocks_per_core
    g.BT = cfg.N_pad // P
    g.qctr = 0
    npc = cfg.nodes_per_core
    Np = cfg.N_pad

    nc = bacc.Bacc(trn_type="TRN2", num_swdge_queues=NQ)
    g.nc = nc

    cs, rs = meta["cs"], meta["rs"]
    maxtg = 0
    for d in (cs, rs):
        for grp in d["groups"]:
            for h in (0, 1):
                maxtg = max(maxtg, sum(d["T"][b][h] for b in grp))
    g.maxtg = int(maxtg)

    # ---- params
    def par(name, shape, dt=F32):
        return nc.declare_dram_parameter(name, list(shape), dt, isOutput=False)

    xs0p = par("xs0", [Np, cfg.D])
    g.dirs = {}
    for nm, d in (("cs", cs), ("rs", rs)):
        g.dirs[nm] = dict(d)
        g.dirs[nm]["idx16_d"] = par(f"idx16_{nm}", [128, d["E_flat"] // 16], I16)
        g.dirs[nm]["dstl_d"] = par(f"dstl_{nm}", [128, d["ntiles"]])
    statc = par("statc", [P, g.bpc, 6])     # u0,q0,r0,aggr0,realmask_sh,sigma0
    rmfp = par("realmask_full", [P, g.BT])
    iotap = par("iotabig", [P, g.maxtg, P])
    identp = par("ident", [P, P])
    onesp = par("ones", [P, 2])             # col of ones; col 1 unused
    pbcp = par("pbc", [2 * P, P])           # p0,p1 broadcast tiles
    wallp = par("wall", [4 * P, P])         # f32 weights: P1,P2,P4,P5
    ballp = par("ball", [4 * P, P])          # bias rows replicated to 128 parts
    wallbp = par("wallb", [6 * P, P], BF16)  # bf16 weights: P7..P14
    ballbp = par("ballb", [6 * P, P], BF16)
    out_p = nc.declare_dram_parameter("out", [npc, cfg.D], F32, isOutput=True)

    # ---- internal dram
    def dram(name, shape, dt=F32, shared=False):
        return nc.dram_tensor(name, list(shape), dt,
                              addr_space="Shared" if shared else "Local")

    xs_w = {k: (192 if k == 3 else 256 if k == 6 else cfg.D)
            for k in range(1, 14)}
    xs_dt = {k: (F32 if k <= 5 else BF16) for k in range(1, 14)}
    XS = {k: dram(f"xs{k}", [Np, xs_w[k]], xs_dt[k], shared=True)
          for k in range(1, 14)}
    xsout = {k: dram(f"xso{k}", [npc, xs_w[k]], xs_dt[k]) for k in range(1, 14)}
    g.z_dram = dram("ztab", [Np, ZW])
    h2save = dram("h2save", [npc, cfg.D])
    h5save = dram("h5save", [npc, cfg.D])
    h3tmp = dram("h3tmp", [npc, cfg.D])
    h6tmp = dram("h6tmp", [npc, cfg.D])
    score_sh_d = {i: dram(f"scsh{i}", [P, g.bpc]) for i in (0, 1)}
    score_fl_d = {i: dram(f"scfl{i}", [NCORES * P, g.bpc], shared=True)
                  for i in (0, 1)}

    with tile.TileContext(nc) as tc:
        g.tc = tc
        ctxs = [
            tc.tile_pool(name="const", bufs=1),
            tc.tile_pool(name="nvp", bufs=1),
            tc.tile_pool(name="idxp", bufs=4),
            tc.tile_pool(name="gathp", bufs=4),
            tc.tile_pool(name="ohtp", bufs=4),
            tc.tile_pool(name="evp", bufs=3),
            tc.tile_pool(name="outp", bufs=3),
            tc.tile_pool(name="zstp", bufs=1),
            tc.tile_pool(name="bisp", bufs=1),
            tc.tile_pool(name="psagg", bufs=4, space="PSUM"),
            tc.tile_pool(name="psw", bufs=2, space="PSUM"),
            tc.tile_pool(name="pst", bufs=2, space="PSUM"),
        ]
        cpool, g.sb_nv, g.sb_idx, g.sb_gath, g.sb_oht, g.sb_ev, g.sb_out, \
            g.sb_zst, g.sb_bis, g.ps_agg, g.ps_w, g.ps_t = \
            [c.__enter__() for c in ctxs]

        # ---- constants into sbuf
        def cload(ap_src, shape, tag, dt=F32):
            t = cpool.tile(list(shape), dt, name=tag, tag=tag)
            nc.sync.dma_start(out=t[:], in_=ap_src)
            return t

        g.iota_big = cload(iotap.ap(), [P, g.maxtg, P], "iota")
        g.ident = cload(identp.ap(), [P, P], "ident")
        g.ident_bf = cpool.tile([P, P], BF16, name="identb", tag="identb")
        nc.vector.tensor_copy(out=g.ident_bf[:], in_=g.ident[:])
        ones2 = cload(onesp.ap(), [P, 2], "ones2")
        g.ones_col = ones2[:, 0:1]
        orow = cpool.tile([1, P], F32, name="orow", tag="orow")
        nc.vector.memset(orow[:], 1.0)
        g.ones_row = orow
        statc_t = cload(statc.ap(), [P, g.bpc, 6], "statc")
        u0c = statc_t[:, :, 0]
        q0c = statc_t[:, :, 1]
        r0c = statc_t[:, :, 2]
        aggr0c = statc_t[:, :, 3]
        rm_sh = statc_t[:, :, 4]
        sig0col = statc_t[:, :, 5]
        rm_fl = cload(rmfp.ap(), [P, g.BT], "rmfl")
        pbc_t = cload(pbcp.ap().rearrange("(v p) d -> p v d", p=P), [P, 2, P], "pbc")
        p0bc, p1bc = pbc_t[:, 0, :], pbc_t[:, 1, :]
        wall = cload(wallp.ap().rearrange("(w p) d -> p w d", p=P),
                     [P, 4, P], "wall")
        ball = cload(ballp.ap().rearrange("(w p) d -> p w d", p=P),
                     [P, 4, P], "ball")
        wallb = cload(wallbp.ap().rearrange("(w p) d -> p w d", p=P),
                      [P, 6, P], "wallb", BF16)
        ballb = cload(ballbp.ap().rearrange("(w p) d -> p w d", p=P),
                      [P, 6, P], "ballb", BF16)
        Wt = [wall[:, i, :] for i in range(4)]
        bt = [ball[:, i, :] for i in range(4)]
        Wtb = [wallb[:, i, :] for i in range(6)]
        btb = [ballb[:, i, :] for i in range(6)]

        # one-time scrub of the gather slots so skipped (trailing-negative)
        # slots never feed NaN garbage into matmuls; shape must be the
        # byte-largest user of the tag (f32 elem=192)
        for _ in range(4):
            z = g.sb_gath.tile([P, g.maxtg, 192], F32, name="fg", tag="fg")
            nc.vector.memset(z[:], 0.0)

        # persistent z-derived columns (filled at P4/P7 evictions)
        sig1col = cpool.tile([P, g.bpc], F32, name="sig1c", tag="sig1c")
        sig2col = cpool.tile([P, g.bpc], F32, name="sig2c", tag="sig2c")
        A1col = cpool.tile([P, g.bpc], F32, name="A1c", tag="A1c")

        def vmul(a, b_, tag):
            t = nv(g, tag)
            nc.vector.tensor_tensor(out=t[:], in0=a[:], in1=b_[:], op=OP.mult)
            return t

        u0u0 = vmul(u0c, u0c, "u0u0")
        u0q0 = vmul(u0c, q0c, "u0q0")

        def scoped(name):
            return nc.named_scope(name)

        def emit_schedule():
            # =========== DOWN LEVEL 0 ===========
            with scoped("P1"):
                emit_dir_pass(g, "cs", xs0p, "conv",
                              conv_block_fn(g, Wt[0], bt[0],
                                            [("xs", xsout[1], u0u0)],
                                            sigma_col=sig0col))
                allgather(g, xsout[1], XS[1])
            with scoped("P2"):
                emit_dir_pass(g, "cs", XS[1], "conv",
                              conv_block_fn(g, Wt[1], bt[1],
                                            [("xs", xsout[2], u0q0),
                                             ("xs", h2save, u0c)],
                                            sigma_col=sig0col))
                allgather(g, xsout[2], XS[2])
            with scoped("P3"):
                score0 = nv(g, "score0")
                emit_dir_pass(g, "cs", XS[2], "wec",
                              wec_block_fn(g, r0c, [("xs", h3tmp, r0c)],
                                           score_to=score0, pbc=p0bc))
                nc.sync.dma_start(out=score_sh_d[0].ap(), in_=score0[:])
                allgather(g, score_sh_d[0], score_fl_d[0])
            with scoped("topk0"):
                sc0f = load_full_from_ag(g, score_fl_d[0], "sc0f")
                sel0f = sel_from(g, sc0f, rm_fl, "sel0f")
                k0 = math.ceil(cfg.ratio * cfg.N)
                thr0 = bisect_topk(g, sel0f, k0, "thr0")
                kept0f = nv(g, "kept0f", [P, g.BT])
                nc.vector.tensor_scalar(out=kept0f[:], in0=sel0f[:],
                                        scalar1=thr0[:], scalar2=None,
                                        op0=OP.is_ge)
                sel0s = sel_from(g, score0, rm_sh, "sel0s")
                kept0s = nv(g, "kept0s")
                nc.vector.tensor_scalar(out=kept0s[:], in0=sel0s[:],
                                        scalar1=thr0[:], scalar2=None,
                                        op0=OP.is_ge)
                tanh0 = nv(g, "tanh0")
                nc.scalar.activation(out=tanh0[:], in_=score0[:], func=AF.Tanh)
            # Z pass A: deg1 raw (rs direction, gather kept0 at col, segsum by row)
            with scoped("ZA"):
                zbuild(g, [kept0f])
                S1 = nv(g, "S1")
                emit_dir_pass(g, "rs", None, "z",
                              z_block_fn(g, [], [(S1, 0)], 1), zcols=1)
            with scoped("lvl1fac"):
                deg1 = vmul(kept0s, S1, "deg1")
                m1 = nv(g, "m1")
                nc.vector.tensor_scalar(out=m1[:], in0=deg1[:], scalar1=0.0,
                                        scalar2=None, op0=OP.is_gt)
                dsafe = nv(g, "dsafe")
                nc.vector.tensor_scalar(out=dsafe[:], in0=deg1[:], scalar1=1e-30,
                                        scalar2=None, op0=OP.max)
                u1 = nv(g, "u1")
                nc.vector.reciprocal(out=u1[:], in_=dsafe[:])
                nc.scalar.activation(out=u1[:], in_=u1[:], func=AF.Sqrt)
                nc.vector.tensor_tensor(out=u1[:], in0=u1[:], in1=m1[:], op=OP.mult)
                w1 = vmul(aggr0c, kept0s, "w1")
                rdeg1 = nv(g, "rdeg1")
                nc.vector.reciprocal(out=rdeg1[:], in_=dsafe[:])
                q1 = vmul(w1, rdeg1, "q1")
                nc.vector.tensor_tensor(out=q1[:], in0=q1[:], in1=m1[:], op=OP.mult)
                # XS3 = [h3 * tanh0 * u1 | u1 | q1 | pad]
                cv3 = vmul(tanh0, u1, "cv3")
                for b in range(g.bpc):
                    t = g.sb_out.tile([P, P], F32, name="rs3", tag="rs3")
                    nc.sync.dma_start(
                        out=t[:],
                        in_=h3tmp.ap().rearrange("(b p) d -> b p d", p=P)[b])
                    nc.vector.tensor_scalar(out=t[:], in0=t[:],
                                            scalar1=cv3[:, b : b + 1],
                                            scalar2=None, op0=OP.mult)
                    nc.sync.dma_start(
                        out=xsout[3].ap().rearrange("(b p) d -> b p d", p=P)
                        [b][:, 0:P], in_=t[:])
                uq = g.sb_out.tile([P, g.bpc, 2], F32, name="uq", tag="uq")
                nc.vector.tensor_copy(out=uq[:, :, 0:1], in_=u1[:, :, None])
                nc.vector.tensor_copy(out=uq[:, :, 1:2], in_=q1[:, :, None])
                nc.sync.dma_start(
                    out=xsout[3].ap().rearrange("(b p) d -> p b d", p=P)
                    [:, :, P : P + 2], in_=uq[:])
                allgather(g, xsout[3], XS[3])

            # =========== DOWN LEVEL 1 ===========
            u1u1 = vmul(u1, u1, "u1u1")
            u1q1 = vmul(u1, q1, "u1q1")
            with scoped("P4"):
                emit_dir_pass(
                    g, "cs", XS[3], "conv",
                    conv_block_fn(g, Wt[2], bt[2], [("xs", xsout[4], u1u1)],
                                  zinfo={"zc": 2,
                                         "cols": [(sig1col, 0), (A1col, 1)],
                                         "bias_col": (sig1col, 0)}),
                    elem=192, zmm=2)
                allgather(g, xsout[4], XS[4])
            with scoped("lvl1fac2"):
                aggr1 = vmul(kept0s, A1col, "aggr1")
                nc.vector.tensor_scalar(out=aggr1[:], in0=aggr1[:], scalar1=1e-12,
                                        scalar2=None, op0=OP.add)
                raggr1 = nv(g, "raggr1")
                nc.vector.reciprocal(out=raggr1[:], in_=aggr1[:])
                r1 = vmul(kept0s, raggr1, "r1")
            with scoped("P5"):
                emit_dir_pass(g, "cs", XS[4], "conv",
                              conv_block_fn(g, Wt[3], bt[3],
                                            [("xs", xsout[5], u1q1),
                                             ("xs", h5save, u1)],
                                            sigma_col=sig1col))
                allgather(g, xsout[5], XS[5])
            with scoped("P6"):
                score1 = nv(g, "score1")
                emit_dir_pass(g, "cs", XS[5], "wec",
                              wec_block_fn(g, r1, [("xs", h6tmp, r1)],
                                           score_to=score1, pbc=p1bc))
                nc.sync.dma_start(out=score_sh_d[1].ap(), in_=score1[:])
                allgather(g, score_sh_d[1], score_fl_d[1])
            with scoped("topk1"):
                sc1f = load_full_from_ag(g, score_fl_d[1], "sc1f")
                sel1f = sel_from(g, sc1f, kept0f, "sel1f")
                k0 = math.ceil(cfg.ratio * cfg.N)
                k1 = math.ceil(cfg.ratio * k0)
                thr1 = bisect_topk(g, sel1f, k1, "thr1")
                kept1f = nv(g, "kept1f", [P, g.BT])
                nc.vector.tensor_scalar(out=kept1f[:], in0=sel1f[:],
                                        scalar1=thr1[:], scalar2=None,
                                        op0=OP.is_ge)
                sel1s = sel_from(g, score1, kept0s, "sel1s")
                kept1s = nv(g, "kept1s")
                nc.vector.tensor_scalar(out=kept1s[:], in0=sel1s[:],
                                        scalar1=thr1[:], scalar2=None,
                                        op0=OP.is_ge)
                tanh1 = nv(g, "tanh1")
                nc.scalar.activation(out=tanh1[:], in_=score1[:], func=AF.Tanh)
            # Z pass C: deg2 raw
            with scoped("ZC"):
                zbuild(g, [kept1f])
                S2 = nv(g, "S2")
                emit_dir_pass(g, "rs", None, "z",
                              z_block_fn(g, [], [(S2, 0)], 1), zcols=1)
            with scoped("lvl2fac"):
                deg2 = vmul(kept1s, S2, "deg2")
                m2 = nv(g, "m2")
                nc.vector.tensor_scalar(out=m2[:], in0=deg2[:], scalar1=0.0,
                                        scalar2=None, op0=OP.is_gt)
                d2safe = nv(g, "d2safe")
                nc.vector.tensor_scalar(out=d2safe[:], in0=deg2[:], scalar1=1e-30,
                                        scalar2=None, op0=OP.max)
                u2 = nv(g, "u2")
                nc.vector.reciprocal(out=u2[:], in_=d2safe[:])
                nc.scalar.activation(out=u2[:], in_=u2[:], func=AF.Sqrt)
                nc.vector.tensor_tensor(out=u2[:], in0=u2[:], in1=m2[:], op=OP.mult)
                # XS6 (bf16) = [h6 * tanh1 * u2 | u2 | pad]
                cv6 = vmul(tanh1, u2, "cv6")
                for b in range(g.bpc):
                    t = g.sb_out.tile([P, P], F32, name="rs6", tag="rs6")
                    nc.sync.dma_start(
                        out=t[:],
                        in_=h6tmp.ap().rearrange("(b p) d -> b p d", p=P)[b])
                    tb = g.sb_out.tile([P, P], BF16, name="rs6b", tag="rs6b")
                    nc.vector.tensor_scalar(out=tb[:], in0=t[:],
                                            scalar1=cv6[:, b : b + 1],
                                            scalar2=None, op0=OP.mult)
                    nc.sync.dma_start(
                        out=xsout[6].ap().rearrange("(b p) d -> b p d", p=P)
                        [b][:, 0:P], in_=tb[:])
                u2b = g.sb_out.tile([P, g.bpc, 1], BF16, name="u2b", tag="u2b")
                nc.vector.tensor_copy(out=u2b[:, :, 0:1], in_=u2[:, :, None])
                nc.sync.dma_start(
                    out=xsout[6].ap().rearrange("(b p) d -> p b d", p=P)
                    [:, :, P : P + 1], in_=u2b[:])
                allgather(g, xsout[6], XS[6])

            # =========== BOTTOM ===========
            u2u2 = vmul(u2, u2, "u2u2")
            u2r1 = vmul(u2, r1, "u2r1")
            with scoped("P7"):
                emit_dir_pass(
                    g, "cs", XS[6], "conv",
                    conv_block_fn(g, Wtb[0], btb[0], [("xs", xsout[7], u2u2)],
                                  dt=BF16,
                                  zinfo={"zc": 1,
                                         "cols": [(sig2col, 0)],
                                         "bias_col": (sig2col, 0)}),
                    elem=256, dt=BF16, zmm=1)
                allgather(g, xsout[7], XS[7])
            with scoped("P8"):
                emit_dir_pass(g, "cs", XS[7], "conv",
                              conv_block_fn(g, Wtb[1], btb[1],
                                            [("xs", xsout[8], u2r1)],
                                            dt=BF16, sigma_col=sig2col),
                              dt=BF16)
                allgather(g, xsout[8], XS[8])

            # =========== UP LEVEL (uses emask1): wec-up + 2 convs ===========
            q1u1 = vmul(q1, u1, "q1u1")
            with scoped("P9"):
                emit_dir_pass(g, "rs", XS[8], "wec",
                              wec_block_fn(g, q1, [("xs", xsout[9], q1u1)]),
                              dt=BF16)
                allgather(g, xsout[9], XS[9])
            with scoped("P10"):
                emit_dir_pass(g, "cs", XS[9], "conv",
                              conv_block_fn(g, Wtb[2], btb[2],
                                            [("xs", xsout[10], u1u1)],
                                            dt=BF16, sigma_col=sig1col),
                              dt=BF16)
                allgather(g, xsout[10], XS[10])
            with scoped("P11"):
                emit_dir_pass(g, "cs", XS[10], "conv",
                              conv_block_fn(g, Wtb[3], btb[3],
                                            [("addshard", h5save, xsout[11],
                                              u1, r0c)],
                                            dt=BF16, sigma_col=sig1col),
                              dt=BF16)
                allgather(g, xsout[11], XS[11])

            # =========== UP LEVEL (emask0) ===========
            q0u0 = vmul(q0c, u0c, "q0u0")
            with scoped("P12"):
                emit_dir_pass(g, "rs", XS[11], "wec",
                              wec_block_fn(g, q0c, [("xs", xsout[12], q0u0)]),
                              dt=BF16)
                allgather(g, xsout[12], XS[12])
            with scoped("P13"):
                emit_dir_pass(g, "cs", XS[12], "conv",
                              conv_block_fn(g, Wtb[4], btb[4],
                                            [("xs", xsout[13], u0u0)],
                                            dt=BF16, sigma_col=sig0col),
                              dt=BF16)
                allgather(g, xsout[13], XS[13])
            with scoped("P14"):
                emit_dir_pass(g, "cs", XS[13], "conv",
                              conv_block_fn(g, Wtb[5], btb[5],
                                            [("addshard", h2save, out_p,
                                              u0c, None)],
                                            dt=BF16, sigma_col=sig0col),
                              dt=BF16)

        emit_schedule()

        for c in reversed(ctxs):
            c.__exit__(None, None, None)

    nc.compile()
    return nc


def make_inmaps(cfg, meta):
    cs, rs = meta["cs"], meta["rs"]
    bpc = cfg.blocks_per_core
    npc = cfg.nodes_per_core
    sv = meta["shardvec"]
    u0s, q0s, r0s, ag0s = (sv(meta[k]) for k in ("u0", "q0", "r0", "aggr0"))
    rms = sv(meta["realmask"])
    sig0s = sv(meta["sigma0"])
    rmf = meta["fullvec"](meta["realmask"])
    maxtg = 0
    for d in (cs, rs):
        for grp in d["groups"]:
            for h in (0, 1):
                maxtg = max(maxtg, sum(d["T"][b][h] for b in grp))
    iota = np.tile(np.arange(P, dtype=np.float32)[None, None, :], (P, maxtg, 1))
    ident = np.eye(P, dtype=np.float32)
    ones = np.ones((P, 2), np.float32)
    pbc = np.concatenate([
        np.tile(meta["p0"][None, :], (P, 1)),
        np.tile(meta["p1"][None, :], (P, 1))], 0).astype(np.float32)

    in_maps = []
    for c in range(NCORES):
        statcv = np.zeros((P, bpc, 6), np.float32)
        statcv[:, :, 0] = u0s[c]
        statcv[:, :, 1] = q0s[c]
        statcv[:, :, 2] = r0s[c]
        statcv[:, :, 3] = ag0s[c]
        statcv[:, :, 4] = rms[c]
        statcv[:, :, 5] = sig0s[c]
        in_maps.append({
            "xs0": meta["xs0"],
            "idx16_cs": cs["idx16"][c], "dstl_cs": cs["dstl"][c],
            "idx16_rs": rs["idx16"][c], "dstl_rs": rs["dstl"][c],
            "statc": statcv,
            "realmask_full": rmf,
            "iotabig": iota, "ident": ident, "ones": ones, "pbc": pbc,
            "wall": None, "ball": None, "wallb": None, "ballb": None,
        })
    return in_maps


def fill_weights(in_maps, Wd, bd, Wu, bu, Wb, bb):
    Wf = [Wd[0, 0], Wd[0, 1], Wd[1, 0], Wd[1, 1]]
    bf = [bd[0, 0], bd[0, 1], bd[1, 0], bd[1, 1]]
    Wb16 = [Wb[0], Wb[1], Wu[0, 0], Wu[0, 1], Wu[1, 0], Wu[1, 1]]
    bb16 = [bb[0], bb[1], bu[0, 0], bu[0, 1], bu[1, 0], bu[1, 1]]
    wall = np.concatenate([w.astype(np.float32) for w in Wf], 0)
    ball = np.concatenate([np.tile(b.astype(np.float32)[None, :], (P, 1))
                           for b in bf], 0)
    wallb = np.concatenate([w for w in Wb16], 0).astype(ml_dtypes.bfloat16)
    ballb = np.concatenate([np.tile(np.asarray(b)[None, :], (P, 1))
                            for b in bb16], 0).astype(ml_dtypes.bfloat16)
    for m in in_maps:
        m["wall"] = wall
        m["ball"] = ball
        m["wallb"] = wallb
        m["ballb"] = ballb


def run_gnn(cfg, inputs, nc_cache={}, full_pad=False, trace=False):
    """Full pipeline: preprocess, build (cached by cfg), run, assemble."""
    x = np.asarray(inputs["x"], np.float32)
    ei = np.asarray(inputs["edge_index"])
    pvec = np.asarray(inputs["pvec"], np.float32)
    meta = preprocess(cfg, x, ei, pvec)
    key = (cfg.N, cfg.E, ei.tobytes()[:64])  # program depends on edge stats
    if key not in nc_cache:
        nc_cache.clear()
        nc_cache[key] = (build_kernel(cfg, meta), None)
    nc, _ = nc_cache[key]
    in_maps = make_inmaps(cfg, meta)
    fill_weights(in_maps, *(np.asarray(inputs[k], np.float32)
                            for k in ("Wd", "bd", "Wu", "bu", "Wb", "bb")))
    res = run_bass_kernel_spmd(nc, in_maps, list(range(NCORES)), trace=trace)
    out = np.concatenate([res.results[c]["out"] for c in range(NCORES)], 0)
    return (out if full_pad else out[: cfg.N]), res


_CFG = Cfg()


def kernel(**inputs):
    out, _ = run_gnn(_CFG, inputs)
    return out.astype(np.float32)


# revision 16
# speedup vs baseline: 1.6494x; 1.0785x over previous
"""Trainium2 Bass kernel for nn_MessagePassingLayer (graph U-Net message
passing) on 8 NeuronCores.

Self-contained: kernel(**inputs) takes the full unsharded inputs and
returns the full [50000, 128] float32 output.

Strategy: nodes padded to 50176 and sharded contiguously over the 8
cores; edges bucketed per (dst block, src half) in destination-sorted
order. Every per-edge coefficient in this network factorizes as
rowfac[row[e]] * colfac[col[e]], so row factors are pre-applied to the
gathered node table (per-pass XS buffers, exchanged via AllGather) and
col factors are applied per-partition on PSUM eviction.

Perf structure (v2):
 - dma_gather calls round-robin over 4 SWDGE queues (desc gen runs on
   distinct Q7 core pairs in parallel).
 - passes P7..P14 (bottom + up path, after both top-k selections) run
   with bf16 gather tables and bf16 matmuls; P1..P6 stay f32 so the
   top-k thresholds match the reference bit-for-bit.
 - the sigma1/A1 and sigma2 z-passes are folded into P4/P7 as extra
   gathered columns + a second per-tile matmul into a spare PSUM
   region; only the two deg z-passes (rs direction) remain.
 - trailing padding slots of each gather call use idx=-1 (descriptor
   emission skipped by the Q7 ucode).
"""
import math
import os

import numpy as np
import ml_dtypes
import concourse.bacc as bacc
import concourse.mybir as mybir
import concourse.tile as tile
from concourse.bass_utils import run_bass_kernel_spmd

from dataclasses import dataclass, field

P = 128
NCORES = 8
NQ = 4            # SWDGE queues used round-robin for gathers


@dataclass
class Cfg:
    N: int = 50000
    E: int = 800000
    D: int = 128
    L: int = 2
    B: int = 2
    ratio: float = 0.5
    # derived
    blocks_per_core: int = field(init=False)
    N_pad: int = field(init=False)
    nodes_per_core: int = field(init=False)
    half: int = field(init=False)          # rows per gather half-table

    def __post_init__(self):
        blocks_total = math.ceil(self.N / P)
        self.blocks_per_core = math.ceil(blocks_total / NCORES)
        self.N_pad = self.blocks_per_core * NCORES * P
        self.nodes_per_core = self.blocks_per_core * P
        self.half = self.N_pad // 2
        assert self.half <= 32768, "gather half-table must fit int16 index"
        assert self.N_pad // 2 % P == 0


def wrap_idx(idx, n):
    """[n] int -> [128, n/16] int16 wrapped+replicated layout for dma_gather."""
    assert n % 16 == 0
    w = np.zeros((16, n // 16), np.int16)
    w[np.arange(n) % 16, np.arange(n) // 16] = idx.astype(np.int16)
    return np.tile(w, (8, 1))


def build_direction(cfg, src, dst, group_blocks):
    """Static tables for one scatter direction.

    src/dst: [E] global endpoint arrays (gather at src, scatter to dst).
    Edges are bucketed per (core, local dst block, src half), padded to
    tiles of 128 with null edges. Tile counts per (block position, half)
    are maxed across cores so the 8 cores share one instruction stream.
    Null slots that end up at the tail of a gather call get idx=-1 (the
    gather ucode skips trailing negative indices); interior nulls gather
    the always-zero pad row.
    """
    bpc = cfg.blocks_per_core
    npc = cfg.nodes_per_core
    half = cfg.half
    zero_row = cfg.N_pad - 1          # a pad node: always-zero row (half 1)

    buckets = [[[None, None] for _ in range(bpc)] for _ in range(NCORES)]
    core_of = dst // npc
    blk_of = (dst % npc) // P
    half_of = (src >= half).astype(np.int64)
    order = np.lexsort((src, half_of, blk_of, core_of))
    key = ((core_of[order] * bpc) + blk_of[order]) * 2 + half_of[order]
    bounds = np.searchsorted(key, np.arange(NCORES * bpc * 2 + 1))
    for c in range(NCORES):
        for b in range(bpc):
            for h in (0, 1):
                kk = (c * bpc + b) * 2 + h
                s, e = bounds[kk], bounds[kk + 1]
                buckets[c][b][h] = order[s:e]

    T = np.zeros((bpc, 2), np.int64)
    for b in range(bpc):
        for h in (0, 1):
            mx = max(len(buckets[c][b][h]) for c in range(NCORES))
            T[b, h] = max(1, math.ceil(mx / P))

    groups = []
    for g0 in range(0, bpc, group_blocks):
        groups.append(list(range(g0, min(g0 + group_blocks, bpc))))

    tot_tiles = int(T.sum())
    E_flat = tot_tiles * P
    idxs = np.zeros((NCORES, E_flat), np.int64)
    dstl = np.zeros((NCORES, E_flat), np.float32)
    pos = 0
    tile_plan = []   # (group, h, b, ntiles, start_pos) shared across cores
    for grp in groups:
        for h in (0, 1):
            for gi, b in enumerate(grp):
                nt = int(T[b, h])
                last_in_call = gi == len(grp) - 1
                tile_plan.append((h, b, nt, pos))
                for c in range(NCORES):
                    ed = buckets[c][b][h]
                    n = len(ed)
                    assert n <= nt * P
                    sl = slice(pos, pos + n)
                    idxs[c, sl] = src[ed] - h * half
                    dstl[c, sl] = (dst[ed] % npc) % P
                    if n < nt * P:
                        psl = slice(pos + n, pos + nt * P)
                        dstl[c, psl] = 200.0       # never matches iota
                        idxs[c, psl] = (zero_row - half) if h == 1 else 0
                pos += nt * P
    assert pos == E_flat

    idx16 = np.stack([wrap_idx(idxs[c], E_flat) for c in range(NCORES)])
    ntiles = E_flat // P
    dstl_t = dstl.reshape(NCORES, ntiles, P).transpose(0, 2, 1).copy()

    return {
        "idx16": idx16,            # [NCORES, 128, E_flat/16] int16
        "dstl": dstl_t,            # [NCORES, 128, ntiles] f32
        "tile_plan": tile_plan,    # shared: (h, b, ntiles, start_pos)
        "groups": groups,
        "E_flat": E_flat,
        "ntiles": ntiles,
        "T": T,
    }


def preprocess(cfg, x, edge_index, pvec):
    """All static host work. Returns per-core input maps pieces + meta."""
    N, Np = cfg.N, cfg.N_pad
    row = edge_index[0].astype(np.int64)
    col = edge_index[1].astype(np.int64)

    deg0 = np.zeros(Np, np.float32)
    np.add.at(deg0, row, 1.0)
    with np.errstate(divide="ignore"):
        dis0 = np.where(deg0 > 0, deg0.astype(np.float64) ** -0.5, 0.0
                        ).astype(np.float32)
        normed0 = np.where(deg0 > 0, 1.0 / np.where(deg0 > 0, deg0, 1), 0.0
                           ).astype(np.float32)
    A0 = np.zeros(Np, np.float32)
    np.add.at(A0, col, normed0[row])
    aggr0 = (A0 + 1e-12).astype(np.float32)
    r0 = (1.0 / aggr0).astype(np.float32)
    q0 = normed0
    u0 = dis0
    sigma0 = np.zeros(Np, np.float32)
    np.add.at(sigma0, col, u0[row])

    cs = build_direction(cfg, row, col, group_blocks=2)
    rs = build_direction(cfg, col, row, group_blocks=2)

    xs0 = np.zeros((Np, cfg.D), np.float32)
    xs0[:N] = x * u0[:N, None]

    realmask = np.zeros(Np, np.float32)
    realmask[:N] = 1.0

    p0 = pvec[0] / np.linalg.norm(pvec[0])
    p1 = pvec[1] / np.linalg.norm(pvec[1])

    def shardvec(v):   # [Np] -> [NCORES, 128, bpc] (partition-major per block)
        return v.reshape(NCORES, cfg.blocks_per_core, P).transpose(0, 2, 1).copy()

    def fullvec(v):    # [Np] -> [128, blocks_total]
        return v.reshape(-1, P).T.copy()

    meta = {
        "cs": cs, "rs": rs,
        "u0": u0, "q0": q0, "r0": r0, "sigma0": sigma0, "aggr0": aggr0,
        "xs0": xs0, "realmask": realmask, "p0": p0, "p1": p1,
        "shardvec": shardvec, "fullvec": fullvec,
    }
    return meta


F32 = mybir.dt.float32
BF16 = mybir.dt.bfloat16
I16 = mybir.dt.int16
AF = mybir.ActivationFunctionType
OP = mybir.AluOpType
AX = mybir.AxisListType

ZCHUNK = 40
ZW = 64
BISECT_ITERS = 34
BISECT_RANGE = 1024.0    # |score| << 512; threshold is within hi-1024..hi


class G:
    """build-time globals bag"""
    pass


# ------------------------------------------------------------- edge passes --

def emit_dir_pass(g, dirn, src_dram, mode, block_fn, zcols=0,
                  elem=128, dt=F32, zmm=0):
    """One edge pass.

    mode: 'conv' (psum [f,d]), 'wec' (psum [d,f]), 'z' (psum [zcols,d]).
    elem: gathered row width in dt elements; cols [P, P+zmm) are per-src
    scalars accumulated into psum region [0:zmm, P:2P] (conv mode only).
    """
    nc = g.nc
    d = g.dirs[dirn]
    if mode == "z":
        elem = ZW
        in_aps = [g.z_dram.ap()[0 : g.half, :], g.z_dram.ap()[g.half :, :]]
    else:
        in_aps = [src_dram.ap()[0 : g.half, :], src_dram.ap()[g.half :, :]]

    plan = {(h, b): (nt, pos) for (h, b, nt, pos) in d["tile_plan"]}
    psums = {}
    done = {}
    for grp in d["groups"]:
        parts = {}
        for h in (0, 1):
            t0 = plan[(h, grp[0])][1] // P
            ntg = sum(plan[(h, b)][0] for b in grp)
            nidx = ntg * P
            it = g.sb_idx.tile([128, nidx // 16], I16, name="idx", tag="idx")
            nc.sync.dma_start(
                out=it[:], in_=d["idx16_d"].ap()[:, t0 * 8 : t0 * 8 + nidx // 16])
            gt = g.sb_gath.tile([P, ntg, elem], dt, name="fg", tag="fg")
            nc.gpsimd.dma_gather(
                out_ap=gt[:], in_ap=in_aps[h], idxs_ap=it[:],
                num_idxs=nidx, num_idxs_reg=nidx, elem_size=elem,
                single_packet=False, queue_num=g.qctr % NQ)
            g.qctr += 1
            dl = g.sb_idx.tile([P, ntg], F32, name="dl", tag="dl")
            nc.sync.dma_start(out=dl[:], in_=d["dstl_d"].ap()[:, t0 : t0 + ntg])
            oh = g.sb_oht.tile([P, ntg, P], dt, name="oht", tag="oht")
            nc.vector.tensor_tensor(
                out=oh[:], in0=dl[:, :, None].to_broadcast([P, ntg, P]),
                in1=g.iota_big[:, : ntg, :], op=OP.is_equal)
            parts[h] = (gt, oh, t0)
        for h in (0, 1):
            gt, oh, t0 = parts[h]
            for b in grp:
                nt, pos = plan[(h, b)]
                rel = pos // P - t0
                if b not in psums:
                    if mode == "z":
                        pp = [zcols, P]
                    elif zmm:
                        pp = [P, 2 * P]
                    else:
                        pp = [P, P]
                    psums[b] = g.ps_agg.tile(pp, F32, space="PSUM", name="agg",
                                             tag="agg")
                    done[b] = 0
                tot = plan[(0, b)][0] + plan[(1, b)][0]
                for t in range(nt):
                    done[b] += 1
                    first, last = done[b] == 1, done[b] == tot
                    if mode == "conv":
                        nc.tensor.matmul(out=psums[b][:, 0:P],
                                         lhsT=gt[:, rel + t, 0:P],
                                         rhs=oh[:, rel + t, :],
                                         start=first, stop=last)
                        if zmm:
                            nc.tensor.matmul(out=psums[b][0:zmm, P : 2 * P],
                                             lhsT=gt[:, rel + t, P : P + zmm],
                                             rhs=oh[:, rel + t, :],
                                             start=first, stop=last)
                    elif mode == "wec":
                        nc.tensor.matmul(out=psums[b][:, 0:P],
                                         lhsT=oh[:, rel + t, :],
                                         rhs=gt[:, rel + t, 0:P],
                                         start=first, stop=last)
                    else:
                        nc.tensor.matmul(out=psums[b][:],
                                         lhsT=gt[:, rel + t, :zcols],
                                         rhs=oh[:, rel + t, :],
                                         start=first, stop=last)
                    if last:
                        block_fn(b, psums[b])
                        del psums[b], done[b]


def conv_block_fn(g, W_sb, b_bc, outs, dt=F32, sigma_col=None, zinfo=None):
    """Per-block eviction for conv passes.

    Bias is applied on DVE at eviction: out += b_bc * (sigma[d] * scale[d]).
    sigma_col: [P, bpc] per-dst sigma shard (used when zinfo is None).
    zinfo: dict(zc=n, cols=[(col_tile, j), ...], bias_col=(tile, j)) --
    sigma/z data come from the pass's own PSUM z region [0:zc, P:2P],
    transposed per block into column tiles.
    """
    nc = g.nc
    ident = g.ident if dt == F32 else g.ident_bf

    def fn(b, pag):
        a1 = g.sb_ev.tile([P, P], dt, name="a1", tag="a1")
        if dt == BF16:
            nc.scalar.activation(out=a1[:], in_=pag[:, 0:P], func=AF.Copy)
        else:
            nc.vector.tensor_copy(out=a1[:], in_=pag[:, 0:P])
        if zinfo is not None:
            zc = zinfo["zc"]
            sigz = g.sb_ev.tile([2, P], dt, name="sigz", tag="sigz")
            nc.vector.tensor_copy(out=sigz[:zc, :], in_=pag[0:zc, P : 2 * P])
            pz = g.ps_t.tile([P, P], dt, space="PSUM", name="pst", tag="pst")
            nc.tensor.transpose(out=pz[:, 0:zc], in_=sigz[0:zc, :],
                                identity=ident[0:zc, 0:zc])
            for (ct, j) in zinfo["cols"]:
                nc.vector.tensor_copy(out=ct[:, b : b + 1], in_=pz[:, j : j + 1])
            bt, bj = zinfo["bias_col"]
            bias_col = bt[:, b : b + 1]
        else:
            bias_col = sigma_col[:, b : b + 1]
        p2 = g.ps_w.tile([P, P], F32, space="PSUM", name="p2", tag="p2")
        nc.tensor.matmul(out=p2[:], lhsT=W_sb[:], rhs=a1[:], start=True,
                         stop=True)
        a2 = g.sb_ev.tile([P, P], dt, name="a2", tag="a2")
        if dt == BF16:
            nc.scalar.activation(out=a2[:], in_=p2[:], func=AF.Copy)
        else:
            nc.vector.tensor_copy(out=a2[:], in_=p2[:])
        p3 = g.ps_t.tile([P, P], dt, space="PSUM", name="pst", tag="pst")
        nc.tensor.transpose(out=p3[:], in_=a2[:], identity=ident[:])
        emit_evictions(g, b, p3, outs, bias_bc=b_bc, bias_col=bias_col)
    return fn


def wec_block_fn(g, colfac, outs, score_to=None, pbc=None):
    nc = g.nc

    def fn(b, pag):
        if score_to is not None:
            sc = g.sb_ev.tile([P, P], F32, name="scm", tag="scm")
            nc.vector.tensor_tensor(out=sc[:], in0=pag[:, 0:P], in1=pbc[:],
                                    op=OP.mult)
            red = g.sb_ev.tile([P, 1], F32, name="scr", tag="scr")
            nc.vector.reduce_sum(red[:], sc[:], axis=AX.X)
            nc.vector.tensor_tensor(out=score_to[:, b : b + 1], in0=red[:],
                                    in1=colfac[:, b : b + 1], op=OP.mult)
        emit_evictions(g, b, pag, outs)
    return fn


def z_block_fn(g, row_to, col_to, zcols):
    """row_to: [(rowtile, j)] copy psum row j; col_to: [(coltile, j)]."""
    nc = g.nc

    def fn(b, pag):
        az = g.sb_ev.tile([P, P], F32, name="az", tag="az")
        nc.vector.tensor_copy(out=az[:zcols, :], in_=pag[:])
        for (rt, j) in row_to:
            nc.vector.tensor_copy(out=rt[:, b * P : (b + 1) * P],
                                  in_=az[j : j + 1, :])
        if col_to:
            pz = g.ps_t.tile([P, P], F32, space="PSUM", name="pst", tag="pst")
            nc.tensor.transpose(out=pz[:, :zcols], in_=az[:zcols, :],
                                identity=g.ident[:zcols, :zcols])
            for (ct, j) in col_to:
                nc.vector.tensor_copy(out=ct[:, b : b + 1],
                                      in_=pz[:, j : j + 1])
    return fn


def emit_evictions(g, b, psum, outs, bias_bc=None, bias_col=None):
    nc = g.nc
    for o in outs:
        kind = o[0]
        if kind == "xs":
            _, dram, scalevec = o
            dt_out = dram.dtype
            if bias_bc is not None:
                t = g.sb_out.tile([P, P], F32, name="xso", tag="xso")
                nc.scalar.activation(out=t[:], in_=psum[:, 0:P], func=AF.Copy,
                                     scale=scalevec[:, b : b + 1])
                to = g.sb_out.tile([P, P], dt_out, name="xso2", tag="xso2")
                sc2 = g.sb_out.tile([P, 1], F32, name="sc2", tag="sc2")
                nc.vector.tensor_tensor(out=sc2[:], in0=bias_col[:],
                                        in1=scalevec[:, b : b + 1], op=OP.mult)
                bt2 = g.sb_out.tile([P, P], F32, name="bt2", tag="bt2")
                nc.vector.tensor_tensor(out=bt2[:], in0=bias_bc[:],
                                        in1=sc2[:].to_broadcast([P, P]),
                                        op=OP.mult)
                nc.vector.tensor_tensor(out=to[:], in0=t[:], in1=bt2[:],
                                        op=OP.add)
            else:
                to = g.sb_out.tile([P, P], dt_out, name="xso2", tag="xso2")
                nc.scalar.activation(out=to[:], in_=psum[:, 0:P], func=AF.Copy,
                                     scale=scalevec[:, b : b + 1])
            nc.sync.dma_start(
                out=dram.ap().rearrange("(b p) d -> b p d", p=P)[b], in_=to[:])
        elif kind == "addshard":
            _, dram_in, dram_out, pre, post = o
            dt_out = dram_out.dtype
            t = g.sb_out.tile([P, P], F32, name="aso", tag="aso")
            nc.scalar.activation(out=t[:], in_=psum[:, 0:P], func=AF.Copy,
                                 scale=pre[:, b : b + 1])
            if bias_bc is not None:
                sc2 = g.sb_out.tile([P, 1], F32, name="sc2", tag="sc2")
                nc.vector.tensor_tensor(out=sc2[:], in0=bias_col[:],
                                        in1=pre[:, b : b + 1], op=OP.mult)
                bt2 = g.sb_out.tile([P, P], F32, name="bt2", tag="bt2")
                nc.vector.tensor_tensor(out=bt2[:], in0=bias_bc[:],
                                        in1=sc2[:].to_broadcast([P, P]),
                                        op=OP.mult)
                nc.vector.tensor_tensor(out=t[:], in0=t[:], in1=bt2[:],
                                        op=OP.add)
            sk = g.sb_out.tile([P, P], F32, name="skl", tag="skl")
            nc.sync.dma_start(
                out=sk[:], in_=dram_in.ap().rearrange("(b p) d -> b p d", p=P)[b])
            to = g.sb_out.tile([P, P], dt_out, name="aso2", tag="aso2")
            if post is not None:
                nc.vector.tensor_tensor(out=t[:], in0=t[:], in1=sk[:], op=OP.add)
                nc.vector.tensor_tensor(out=to[:], in0=t[:],
                                        in1=post[:, b : b + 1].to_broadcast([P, P]),
                                        op=OP.mult)
            else:
                nc.vector.tensor_tensor(out=to[:], in0=t[:], in1=sk[:], op=OP.add)
            nc.sync.dma_start(
                out=dram_out.ap().rearrange("(b p) d -> b p d", p=P)[b], in_=to[:])


# ------------------------------------------------------------- small pieces --

def allgather(g, in_dram, out_dram):
    g.nc.gpsimd.collective_compute(
        "AllGather", OP.bypass, replica_groups=[list(range(NCORES))],
        ins=[in_dram.ap()], outs=[out_dram.ap()])


def zbuild(g, cols):
    nc = g.nc
    for c0 in range(0, g.BT, ZCHUNK):
        nb = min(ZCHUNK, g.BT - c0)
        st = g.sb_zst.tile([P, ZCHUNK, ZW], F32, name="zst", tag="zst")
        for j, v in enumerate(cols):
            nc.vector.tensor_copy(out=st[:, :nb, j : j + 1],
                                  in_=v[:, c0 : c0 + nb, None])
        nc.sync.dma_start(
            out=g.z_dram.ap().rearrange("(b p) w -> p b w", p=P)[:, c0 : c0 + nb, :],
            in_=st[:, :nb, :])


def cross_part(g, col, op):
    nc = g.nc
    if op == "sum":
        pc = g.ps_t.tile([P, P], F32, space="PSUM", name="pst", tag="pst")
        nc.tensor.matmul(out=pc[:1, :1], lhsT=col[:], rhs=g.ones_col[:],
                         start=True, stop=True)
        out = g.sb_bis.tile([1, 1], F32, name="cnt", tag="cnt")
        nc.vector.tensor_copy(out=out[:], in_=pc[:1, :1])
        return out
    pt = g.ps_t.tile([P, P], F32, space="PSUM", name="pst", tag="pst")
    nc.tensor.transpose(out=pt[:1, :], in_=col[:], identity=g.ident[:])
    row = g.sb_bis.tile([1, P], F32, name="brow", tag="brow")
    nc.vector.tensor_copy(out=row[:], in_=pt[:1, :])
    out = g.sb_bis.tile([1, 1], F32, name="bred", tag="bred")
    nc.vector.reduce_max(out[:], row[:], axis=AX.X)
    return out


def bcast_scalar(g, s11, tag):
    nc = g.nc
    pb = g.ps_t.tile([P, P], F32, space="PSUM", name="pst", tag="pst")
    nc.tensor.matmul(out=pb[:, :1], lhsT=g.ones_row[:], rhs=s11[:],
                     start=True, stop=True)
    out = g.sb_nv.tile([P, 1], F32, name=tag, tag=tag)
    nc.vector.tensor_copy(out=out[:], in_=pb[:, :1])
    return out


def bisect_topk(g, sel_full, k, tag):
    """threshold col [128,1] such that count(sel >= t) == k exactly."""
    nc = g.nc
    mx = g.sb_bis.tile([P, 1], F32, name="bmx", tag="bmx")
    nc.vector.reduce_max(mx[:], sel_full[:], axis=AX.X)
    hi = cross_part(g, mx, "max")
    nc.vector.tensor_scalar(out=hi[:], in0=hi[:], scalar1=1.0, scalar2=None,
                            op0=OP.add)
    lo = g.sb_bis.tile([1, 1], F32, name="blo", tag="blo")
    nc.vector.tensor_scalar(out=lo[:], in0=hi[:], scalar1=-BISECT_RANGE,
                            scalar2=None, op0=OP.add)
    t = g.sb_bis.tile([1, 1], F32, name="bt", tag="bt")
    for _ in range(BISECT_ITERS):
        nc.vector.tensor_tensor(out=t[:], in0=lo[:], in1=hi[:], op=OP.add)
        nc.vector.tensor_scalar(out=t[:], in0=t[:], scalar1=0.5, scalar2=None,
                                op0=OP.mult)
        tcol = bcast_scalar(g, t, "btc")
        cmp = g.sb_bis.tile([P, g.BT], F32, name="bcmp", tag="bcmp")
        nc.vector.tensor_tensor(out=cmp[:], in0=sel_full[:],
                                in1=tcol[:].to_broadcast([P, g.BT]),
                                op=OP.is_ge)
        red = g.sb_bis.tile([P, 1], F32, name="bred2", tag="bred2")
        nc.vector.reduce_sum(red[:], cmp[:], axis=AX.X)
        cnt = cross_part(g, red, "sum")
        flag = g.sb_bis.tile([1, 1], F32, name="bflag", tag="bflag")
        nc.vector.tensor_scalar(out=flag[:], in0=cnt[:], scalar1=float(k) - 0.5,
                                scalar2=None, op0=OP.is_ge)
        d1 = g.sb_bis.tile([1, 1], F32, name="bd1", tag="bd1")
        nc.vector.tensor_tensor(out=d1[:], in0=t[:], in1=lo[:], op=OP.subtract)
        nc.vector.tensor_tensor(out=d1[:], in0=d1[:], in1=flag[:], op=OP.mult)
        nc.vector.tensor_tensor(out=lo[:], in0=lo[:], in1=d1[:], op=OP.add)
        nf = g.sb_bis.tile([1, 1], F32, name="bnf", tag="bnf")
        nc.vector.tensor_scalar(out=nf[:], in0=flag[:], scalar1=-1.0, scalar2=1.0,
                                op0=OP.mult, op1=OP.add)
        d2 = g.sb_bis.tile([1, 1], F32, name="bd2", tag="bd2")
        nc.vector.tensor_tensor(out=d2[:], in0=t[:], in1=hi[:], op=OP.subtract)
        nc.vector.tensor_tensor(out=d2[:], in0=d2[:], in1=nf[:], op=OP.mult)
        nc.vector.tensor_tensor(out=hi[:], in0=hi[:], in1=d2[:], op=OP.add)
    return bcast_scalar(g, lo, tag)


def load_full_from_ag(g, ag_dram, tag, nvec=1, vec=0):
    """AG out dram [(8*nvec*128), bpc] -> [128, BT] sbuf."""
    nc = g.nc
    out = g.sb_nv.tile([P, g.BT], F32, name=tag, tag=tag)
    for r in range(NCORES):
        src = ag_dram.ap().rearrange("(r v p) b -> r v p b", v=nvec, p=P)[r, vec]
        nc.sync.dma_start(out=out[:, r * g.bpc : (r + 1) * g.bpc], in_=src)
    return out


def nv(g, tag, shape=None):
    return g.sb_nv.tile(shape or [P, g.bpc], F32, name=tag, tag=tag)


def sel_from(g, score, active, tag):
    """sel = score*active + (active-1)*1e30 (elementwise, any width)."""
    nc = g.nc
    t1 = nv(g, tag, [P, score.shape[-1]])
    nc.vector.tensor_tensor(out=t1[:], in0=score[:], in1=active[:], op=OP.mult)
    t2 = nv(g, tag + "_m", [P, score.shape[-1]])
    nc.vector.tensor_scalar(out=t2[:], in0=active[:], scalar1=1e30,
                            scalar2=-1e30, op0=OP.mult, op1=OP.add)
    nc.vector.tensor_tensor(out=t1[:], in0=t1[:], in1=t2[:], op=OP.add)
    return t1


def build_kernel(cfg, meta):
    g = G()
    g.D = cfg.D
    g.half = cfg.half
    g.bpc = cfg.blocks_per_core
    g.BT = cfg.N_pad // P
    g.qctr = 0
    npc = cfg.nodes_per_core
    Np = cfg.N_pad

    nc = bacc.Bacc(trn_type="TRN2", num_swdge_queues=NQ)
    g.nc = nc

    cs, rs = meta["cs"], meta["rs"]
    maxtg = 0
    for d in (cs, rs):
        for grp in d["groups"]:
            for h in (0, 1):
                maxtg = max(maxtg, sum(d["T"][b][h] for b in grp))
    g.maxtg = int(maxtg)

    # ---- params
    def par(name, shape, dt=F32):
        return nc.declare_dram_parameter(name, list(shape), dt, isOutput=False)

    xs0p = par("xs0", [Np, cfg.D])
    g.dirs = {}
    for nm, d in (("cs", cs), ("rs", rs)):
        g.dirs[nm] = dict(d)
        g.dirs[nm]["idx16_d"] = par(f"idx16_{nm}", [128, d["E_flat"] // 16], I16)
        g.dirs[nm]["dstl_d"] = par(f"dstl_{nm}", [128, d["ntiles"]])
    statc = par("statc", [P, g.bpc, 6])     # u0,q0,r0,aggr0,realmask_sh,sigma0
    rmfp = par("realmask_full", [P, g.BT])
    iotap = par("iotabig", [P, g.maxtg, P])
    identp = par("ident", [P, P])
    onesp = par("ones", [P, 2])             # col of ones; col 1 unused
    pbcp = par("pbc", [2 * P, P])           # p0,p1 broadcast tiles
    wallp = par("wall", [4 * P, P])         # f32 weights: P1,P2,P4,P5
    ballp = par("ball", [4 * P, P])          # bias rows replicated to 128 parts
    wallbp = par("wallb", [6 * P, P], BF16)  # bf16 weights: P7..P14
    ballbp = par("ballb", [6 * P, P], BF16)
    out_p = nc.declare_dram_parameter("out", [npc, cfg.D], F32, isOutput=True)

    # ---- internal dram
    def dram(name, shape, dt=F32, shared=False):
        return nc.dram_tensor(name, list(shape), dt,
                              addr_space="Shared" if shared else "Local")

    xs_w = {k: (192 if k == 3 else 256 if k == 6 else cfg.D)
            for k in range(1, 14)}
    xs_dt = {k: (F32 if k <= 5 else BF16) for k in range(1, 14)}
    XS = {k: dram(f"xs{k}", [Np, xs_w[k]], xs_dt[k], shared=True)
          for k in range(1, 14)}
    xsout = {k: dram(f"xso{k}", [npc, xs_w[k]], xs_dt[k]) for k in range(1, 14)}
    g.z_dram = dram("ztab", [Np, ZW])
    h2save = dram("h2save", [npc, cfg.D])
    h5save = dram("h5save", [npc, cfg.D])
    h3tmp = dram("h3tmp", [npc, cfg.D])
    h6tmp = dram("h6tmp", [npc, cfg.D])
    score_sh_d = {i: dram(f"scsh{i}", [P, g.bpc]) for i in (0, 1)}
    score_fl_d = {i: dram(f"scfl{i}", [NCORES * P, g.bpc], shared=True)
                  for i in (0, 1)}

    with tile.TileContext(nc) as tc:
        g.tc = tc
        ctxs = [
            tc.tile_pool(name="const", bufs=1),
            tc.tile_pool(name="nvp", bufs=1),
            tc.tile_pool(name="idxp", bufs=4),
            tc.tile_pool(name="gathp", bufs=4),
            tc.tile_pool(name="ohtp", bufs=4),
            tc.tile_pool(name="evp", bufs=3),
            tc.tile_pool(name="outp", bufs=3),
            tc.tile_pool(name="zstp", bufs=1),
            tc.tile_pool(name="bisp", bufs=1),
            tc.tile_pool(name="psagg", bufs=4, space="PSUM"),
            tc.tile_pool(name="psw", bufs=2, space="PSUM"),
            tc.tile_pool(name="pst", bufs=2, space="PSUM"),
        ]
        cpool, g.sb_nv, g.sb_idx, g.sb_gath, g.sb_oht, g.sb_ev, g.sb_out, \
            g.sb_zst, g.sb_bis, g.ps_agg, g.ps_w, g.ps_t = \
            [c.__enter__() for c in ctxs]

        # ---- constants into sbuf
        def cload(ap_src, shape, tag, dt=F32):
            t = cpool.tile(list(shape), dt, name=tag, tag=tag)
            nc.sync.dma_start(out=t[:], in_=ap_src)
            return t

        g.iota_big = cload(iotap.ap(), [P, g.maxtg, P], "iota")
        g.ident = cload(identp.ap(), [P, P], "ident")
        g.ident_bf = cpool.tile([P, P], BF16, name="identb", tag="identb")
        nc.vector.tensor_copy(out=g.ident_bf[:], in_=g.ident[:])
        ones2 = cload(onesp.ap(), [P, 2], "ones2")
        g.ones_col = ones2[:, 0:1]
        orow = cpool.tile([1, P], F32, name="orow", tag="orow")
        nc.vector.memset(orow[:], 1.0)
        g.ones_row = orow
        statc_t = cload(statc.ap(), [P, g.bpc, 6], "statc")
        u0c = statc_t[:, :, 0]
        q0c = statc_t[:, :, 1]
        r0c = statc_t[:, :, 2]
        aggr0c = statc_t[:, :, 3]
        rm_sh = statc_t[:, :, 4]
        sig0col = statc_t[:, :, 5]
        rm_fl = cload(rmfp.ap(), [P, g.BT], "rmfl")
        pbc_t = cload(pbcp.ap().rearrange("(v p) d -> p v d", p=P), [P, 2, P], "pbc")
        p0bc, p1bc = pbc_t[:, 0, :], pbc_t[:, 1, :]
        wall = cload(wallp.ap().rearrange("(w p) d -> p w d", p=P),
                     [P, 4, P], "wall")
        ball = cload(ballp.ap().rearrange("(w p) d -> p w d", p=P),
                     [P, 4, P], "ball")
        wallb = cload(wallbp.ap().rearrange("(w p) d -> p w d", p=P),
                      [P, 6, P], "wallb", BF16)
        ballb = cload(ballbp.ap().rearrange("(w p) d -> p w d", p=P),
                      [P, 6, P], "ballb", BF16)
        Wt = [wall[:, i, :] for i in range(4)]
        bt = [ball[:, i, :] for i in range(4)]
        Wtb = [wallb[:, i, :] for i in range(6)]
        btb = [ballb[:, i, :] for i in range(6)]

        # one-time scrub of the gather slots so skipped (trailing-negative)
        # slots never feed NaN garbage into matmuls; shape must be the
        # byte-largest user of the tag (f32 elem=192)
        for _ in range(4):
            z = g.sb_gath.tile([P, g.maxtg, 192], F32, name="fg", tag="fg")
            nc.vector.memset(z[:], 0.0)

        # persistent z-derived columns (filled at P4/P7 evictions)
        sig1col = cpool.tile([P, g.bpc], F32, name="sig1c", tag="sig1c")
        sig2col = cpool.tile([P, g.bpc], F32, name="sig2c", tag="sig2c")
        A1col = cpool.tile([P, g.bpc], F32, name="A1c", tag="A1c")

        def vmul(a, b_, tag):
            t = nv(g, tag)
            nc.vector.tensor_tensor(out=t[:], in0=a[:], in1=b_[:], op=OP.mult)
            return t

        u0u0 = vmul(u0c, u0c, "u0u0")
        u0q0 = vmul(u0c, q0c, "u0q0")

        def scoped(name):
            return nc.named_scope(name)

        def emit_schedule():
            # =========== DOWN LEVEL 0 ===========
            with scoped("P1"):
                emit_dir_pass(g, "cs", xs0p, "conv",
                              conv_block_fn(g, Wt[0], bt[0],
                                            [("xs", xsout[1], u0u0)],
                                            sigma_col=sig0col))
                allgather(g, xsout[1], XS[1])
            with scoped("P2"):
                emit_dir_pass(g, "cs", XS[1], "conv",
                              conv_block_fn(g, Wt[1], bt[1],
                                            [("xs", xsout[2], u0q0),
                                             ("xs", h2save, u0c)],
                                            sigma_col=sig0col))
                allgather(g, xsout[2], XS[2])
            with scoped("P3"):
                score0 = nv(g, "score0")
                emit_dir_pass(g, "cs", XS[2], "wec",
                              wec_block_fn(g, r0c, [("xs", h3tmp, r0c)],
                                           score_to=score0, pbc=p0bc))
                nc.sync.dma_start(out=score_sh_d[0].ap(), in_=score0[:])
                allgather(g, score_sh_d[0], score_fl_d[0])
            with scoped("topk0"):
                sc0f = load_full_from_ag(g, score_fl_d[0], "sc0f")
                sel0f = sel_from(g, sc0f, rm_fl, "sel0f")
                k0 = math.ceil(cfg.ratio * cfg.N)
                thr0 = bisect_topk(g, sel0f, k0, "thr0")
                kept0f = nv(g, "kept0f", [P, g.BT])
                nc.vector.tensor_tensor(out=kept0f[:], in0=sel0f[:],
                                        in1=thr0[:].to_broadcast([P, g.BT]),
                                        op=OP.is_ge)
                sel0s = sel_from(g, score0, rm_sh, "sel0s")
                kept0s = nv(g, "kept0s")
                nc.vector.tensor_tensor(out=kept0s[:], in0=sel0s[:],
                                        in1=thr0[:].to_broadcast([P, g.bpc]),
                                        op=OP.is_ge)
                tanh0 = nv(g, "tanh0")
                nc.scalar.activation(out=tanh0[:], in_=score0[:], func=AF.Tanh)
            # Z pass A: deg1 raw (rs direction, gather kept0 at col, segsum by row)
            with scoped("ZA"):
                zbuild(g, [kept0f])
                S1 = nv(g, "S1")
                emit_dir_pass(g, "rs", None, "z",
                              z_block_fn(g, [], [(S1, 0)], 1), zcols=1)
            with scoped("lvl1fac"):
                deg1 = vmul(kept0s, S1, "deg1")
                m1 = nv(g, "m1")
                nc.vector.tensor_scalar(out=m1[:], in0=deg1[:], scalar1=0.0,
                                        scalar2=None, op0=OP.is_gt)
                dsafe = nv(g, "dsafe")
                nc.vector.tensor_scalar(out=dsafe[:], in0=deg1[:], scalar1=1e-30,
                                        scalar2=None, op0=OP.max)
                u1 = nv(g, "u1")
                nc.vector.reciprocal(out=u1[:], in_=dsafe[:])
                nc.scalar.activation(out=u1[:], in_=u1[:], func=AF.Sqrt)
                nc.vector.tensor_tensor(out=u1[:], in0=u1[:], in1=m1[:], op=OP.mult)
                w1 = vmul(aggr0c, kept0s, "w1")
                rdeg1 = nv(g, "rdeg1")
                nc.vector.reciprocal(out=rdeg1[:], in_=dsafe[:])
                q1 = vmul(w1, rdeg1, "q1")
                nc.vector.tensor_tensor(out=q1[:], in0=q1[:], in1=m1[:], op=OP.mult)
                # XS3 = [h3 * tanh0 * u1 | u1 | q1 | pad]
                cv3 = vmul(tanh0, u1, "cv3")
                for b in range(g.bpc):
                    t = g.sb_out.tile([P, P], F32, name="rs3", tag="rs3")
                    nc.sync.dma_start(
                        out=t[:],
                        in_=h3tmp.ap().rearrange("(b p) d -> b p d", p=P)[b])
                    nc.vector.tensor_tensor(out=t[:], in0=t[:],
                                            in1=cv3[:, b : b + 1].to_broadcast([P, P]),
                                            op=OP.mult)
                    nc.sync.dma_start(
                        out=xsout[3].ap().rearrange("(b p) d -> b p d", p=P)
                        [b][:, 0:P], in_=t[:])
                uq = g.sb_out.tile([P, g.bpc, 2], F32, name="uq", tag="uq")
                nc.vector.tensor_copy(out=uq[:, :, 0:1], in_=u1[:, :, None])
                nc.vector.tensor_copy(out=uq[:, :, 1:2], in_=q1[:, :, None])
                nc.sync.dma_start(
                    out=xsout[3].ap().rearrange("(b p) d -> p b d", p=P)
                    [:, :, P : P + 2], in_=uq[:])
                allgather(g, xsout[3], XS[3])

            # =========== DOWN LEVEL 1 ===========
            u1u1 = vmul(u1, u1, "u1u1")
            u1q1 = vmul(u1, q1, "u1q1")
            with scoped("P4"):
                emit_dir_pass(
                    g, "cs", XS[3], "conv",
                    conv_block_fn(g, Wt[2], bt[2], [("xs", xsout[4], u1u1)],
                                  zinfo={"zc": 2,
                                         "cols": [(sig1col, 0), (A1col, 1)],
                                         "bias_col": (sig1col, 0)}),
                    elem=192, zmm=2)
                allgather(g, xsout[4], XS[4])
            with scoped("lvl1fac2"):
                aggr1 = vmul(kept0s, A1col, "aggr1")
                nc.vector.tensor_scalar(out=aggr1[:], in0=aggr1[:], scalar1=1e-12,
                                        scalar2=None, op0=OP.add)
                raggr1 = nv(g, "raggr1")
                nc.vector.reciprocal(out=raggr1[:], in_=aggr1[:])
                r1 = vmul(kept0s, raggr1, "r1")
            with scoped("P5"):
                emit_dir_pass(g, "cs", XS[4], "conv",
                              conv_block_fn(g, Wt[3], bt[3],
                                            [("xs", xsout[5], u1q1),
                                             ("xs", h5save, u1)],
                                            sigma_col=sig1col))
                allgather(g, xsout[5], XS[5])
            with scoped("P6"):
                score1 = nv(g, "score1")
                emit_dir_pass(g, "cs", XS[5], "wec",
                              wec_block_fn(g, r1, [("xs", h6tmp, r1)],
                                           score_to=score1, pbc=p1bc))
                nc.sync.dma_start(out=score_sh_d[1].ap(), in_=score1[:])
                allgather(g, score_sh_d[1], score_fl_d[1])
            with scoped("topk1"):
                sc1f = load_full_from_ag(g, score_fl_d[1], "sc1f")
                sel1f = sel_from(g, sc1f, kept0f, "sel1f")
                k0 = math.ceil(cfg.ratio * cfg.N)
                k1 = math.ceil(cfg.ratio * k0)
                thr1 = bisect_topk(g, sel1f, k1, "thr1")
                kept1f = nv(g, "kept1f", [P, g.BT])
                nc.vector.tensor_tensor(out=kept1f[:], in0=sel1f[:],
                                        in1=thr1[:].to_broadcast([P, g.BT]),
                                        op=OP.is_ge)
                sel1s = sel_from(g, score1, kept0s, "sel1s")
                kept1s = nv(g, "kept1s")
                nc.vector.tensor_tensor(out=kept1s[:], in0=sel1s[:],
                                        in1=thr1[:].to_broadcast([P, g.bpc]),
                                        op=OP.is_ge)
                tanh1 = nv(g, "tanh1")
                nc.scalar.activation(out=tanh1[:], in_=score1[:], func=AF.Tanh)
            # Z pass C: deg2 raw
            with scoped("ZC"):
                zbuild(g, [kept1f])
                S2 = nv(g, "S2")
                emit_dir_pass(g, "rs", None, "z",
                              z_block_fn(g, [], [(S2, 0)], 1), zcols=1)
            with scoped("lvl2fac"):
                deg2 = vmul(kept1s, S2, "deg2")
                m2 = nv(g, "m2")
                nc.vector.tensor_scalar(out=m2[:], in0=deg2[:], scalar1=0.0,
                                        scalar2=None, op0=OP.is_gt)
                d2safe = nv(g, "d2safe")
                nc.vector.tensor_scalar(out=d2safe[:], in0=deg2[:], scalar1=1e-30,
                                        scalar2=None, op0=OP.max)
                u2 = nv(g, "u2")
                nc.vector.reciprocal(out=u2[:], in_=d2safe[:])
                nc.scalar.activation(out=u2[:], in_=u2[:], func=AF.Sqrt)
                nc.vector.tensor_tensor(out=u2[:], in0=u2[:], in1=m2[:], op=OP.mult)
                # XS6 (bf16) = [h6 * tanh1 * u2 | u2 | pad]
                cv6 = vmul(tanh1, u2, "cv6")
                for b in range(g.bpc):
                    t = g.sb_out.tile([P, P], F32, name="rs6", tag="rs6")
                    nc.sync.dma_start(
                        out=t[:],
                        in_=h6tmp.ap().rearrange("(b p) d -> b p d", p=P)[b])
                    tb = g.sb_out.tile([P, P], BF16, name="rs6b", tag="rs6b")
                    nc.vector.tensor_tensor(out=tb[:], in0=t[:],
                                            in1=cv6[:, b : b + 1].to_broadcast([P, P]),
                                            op=OP.mult)
                    nc.sync.dma_start(
                        out=xsout[6].ap().rearrange("(b p) d -> b p d", p=P)
                        [b][:, 0:P], in_=tb[:])
                u2b = g.sb_out.tile([P, g.bpc, 1], BF16, name="u2b", tag="u2b")
                nc.vector.tensor_copy(out=u2b[:, :, 0:1], in_=u2[:, :, None])
                nc.sync.dma_start(
                    out=xsout[6].ap().rearrange("(b p) d -> p b d", p=P)
                    [:, :, P : P + 1], in_=u2b[:])
                allgather(g, xsout[6], XS[6])

            # =========== BOTTOM ===========
            u2u2 = vmul(u2, u2, "u2u2")
            u2r1 = vmul(u2, r1, "u2r1")
            with scoped("P7"):
                emit_dir_pass(
                    g, "cs", XS[6], "conv",
                    conv_block_fn(g, Wtb[0], btb[0], [("xs", xsout[7], u2u2)],
                                  dt=BF16,
                                  zinfo={"zc": 1,
                                         "cols": [(sig2col, 0)],
                                         "bias_col": (sig2col, 0)}),
                    elem=256, dt=BF16, zmm=1)
                allgather(g, xsout[7], XS[7])
            with scoped("P8"):
                emit_dir_pass(g, "cs", XS[7], "conv",
                              conv_block_fn(g, Wtb[1], btb[1],
                                            [("xs", xsout[8], u2r1)],
                                            dt=BF16, sigma_col=sig2col),
                              dt=BF16)
                allgather(g, xsout[8], XS[8])

            # =========== UP LEVEL (uses emask1): wec-up + 2 convs ===========
            q1u1 = vmul(q1, u1, "q1u1")
            with scoped("P9"):
                emit_dir_pass(g, "rs", XS[8], "wec",
                              wec_block_fn(g, q1, [("xs", xsout[9], q1u1)]),
                              dt=BF16)
                allgather(g, xsout[9], XS[9])
            with scoped("P10"):
                emit_dir_pass(g, "cs", XS[9], "conv",
                              conv_block_fn(g, Wtb[2], btb[2],
                                            [("xs", xsout[10], u1u1)],
                                            dt=BF16, sigma_col=sig1col),
                              dt=BF16)
                allgather(g, xsout[10], XS[10])
            with scoped("P11"):
                emit_dir_pass(g, "cs", XS[10], "conv",
                              conv_block_fn(g, Wtb[3], btb[3],
                                            [("addshard", h5save, xsout[11],
                                              u1, r0c)],
                                            dt=BF16, sigma_col=sig1col),
                              dt=BF16)
                allgather(g, xsout[11], XS[11])

            # =========== UP LEVEL (emask0) ===========
            q0u0 = vmul(q0c, u0c, "q0u0")
            with scoped("P12"):
                emit_dir_pass(g, "rs", XS[11], "wec",
                              wec_block_fn(g, q0c, [("xs", xsout[12], q0u0)]),
                              dt=BF16)
                allgather(g, xsout[12], XS[12])
            with scoped("P13"):
                emit_dir_pass(g, "cs", XS[12], "conv",
                              conv_block_fn(g, Wtb[4], btb[4],
                                            [("xs", xsout[13], u0u0)],
                                            dt=BF16, sigma_col=sig0col),
                              dt=BF16)
                allgather(g, xsout[13], XS[13])
            with scoped("P14"):
                emit_dir_pass(g, "cs", XS[13], "conv",
                              conv_block_fn(g, Wtb[5], btb[5],
                                            [("addshard", h2save, out_p,
                                              u0c, None)],
                                            dt=BF16, sigma_col=sig0col),
                              dt=BF16)

        emit_schedule()

        for c in reversed(ctxs):
            c.__exit__(None, None, None)

    nc.compile()
    return nc


def make_inmaps(cfg, meta):
    cs, rs = meta["cs"], meta["rs"]
    bpc = cfg.blocks_per_core
    npc = cfg.nodes_per_core
    sv = meta["shardvec"]
    u0s, q0s, r0s, ag0s = (sv(meta[k]) for k in ("u0", "q0", "r0", "aggr0"))
    rms = sv(meta["realmask"])
    sig0s = sv(meta["sigma0"])
    rmf = meta["fullvec"](meta["realmask"])
    maxtg = 0
    for d in (cs, rs):
        for grp in d["groups"]:
            for h in (0, 1):
                maxtg = max(maxtg, sum(d["T"][b][h] for b in grp))
    iota = np.tile(np.arange(P, dtype=np.float32)[None, None, :], (P, maxtg, 1))
    ident = np.eye(P, dtype=np.float32)
    ones = np.ones((P, 2), np.float32)
    pbc = np.concatenate([
        np.tile(meta["p0"][None, :], (P, 1)),
        np.tile(meta["p1"][None, :], (P, 1))], 0).astype(np.float32)

    in_maps = []
    for c in range(NCORES):
        statcv = np.zeros((P, bpc, 6), np.float32)
        statcv[:, :, 0] = u0s[c]
        statcv[:, :, 1] = q0s[c]
        statcv[:, :, 2] = r0s[c]
        statcv[:, :, 3] = ag0s[c]
        statcv[:, :, 4] = rms[c]
        statcv[:, :, 5] = sig0s[c]
        in_maps.append({
            "xs0": meta["xs0"],
            "idx16_cs": cs["idx16"][c], "dstl_cs": cs["dstl"][c],
            "idx16_rs": rs["idx16"][c], "dstl_rs": rs["dstl"][c],
            "statc": statcv,
            "realmask_full": rmf,
            "iotabig": iota, "ident": ident, "ones": ones, "pbc": pbc,
            "wall": None, "ball": None, "wallb": None, "ballb": None,
        })
    return in_maps


def fill_weights(in_maps, Wd, bd, Wu, bu, Wb, bb):
    Wf = [Wd[0, 0], Wd[0, 1], Wd[1, 0], Wd[1, 1]]
    bf = [bd[0, 0], bd[0, 1], bd[1, 0], bd[1, 1]]
    Wb16 = [Wb[0], Wb[1], Wu[0, 0], Wu[0, 1], Wu[1, 0], Wu[1, 1]]
    bb16 = [bb[0], bb[1], bu[0, 0], bu[0, 1], bu[1, 0], bu[1, 1]]
    wall = np.concatenate([w.astype(np.float32) for w in Wf], 0)
    ball = np.concatenate([np.tile(b.astype(np.float32)[None, :], (P, 1))
                           for b in bf], 0)
    wallb = np.concatenate([w for w in Wb16], 0).astype(ml_dtypes.bfloat16)
    ballb = np.concatenate([np.tile(np.asarray(b)[None, :], (P, 1))
                            for b in bb16], 0).astype(ml_dtypes.bfloat16)
    for m in in_maps:
        m["wall"] = wall
        m["ball"] = ball
        m["wallb"] = wallb
        m["ballb"] = ballb


def run_gnn(cfg, inputs, nc_cache={}, full_pad=False, trace=False):
    """Full pipeline: preprocess, build (cached by cfg), run, assemble."""
    x = np.asarray(inputs["x"], np.float32)
    ei = np.asarray(inputs["edge_index"])
    pvec = np.asarray(inputs["pvec"], np.float32)
    meta = preprocess(cfg, x, ei, pvec)
    key = (cfg.N, cfg.E, ei.tobytes()[:64])  # program depends on edge stats
    if key not in nc_cache:
        nc_cache.clear()
        nc_cache[key] = (build_kernel(cfg, meta), None)
    nc, _ = nc_cache[key]
    in_maps = make_inmaps(cfg, meta)
    fill_weights(in_maps, *(np.asarray(inputs[k], np.float32)
                            for k in ("Wd", "bd", "Wu", "bu", "Wb", "bb")))
    res = run_bass_kernel_spmd(nc, in_maps, list(range(NCORES)), trace=trace)
    out = np.concatenate([res.results[c]["out"] for c in range(NCORES)], 0)
    return (out if full_pad else out[: cfg.N]), res


_CFG = Cfg()


def kernel(**inputs):
    out, _ = run_gnn(_CFG, inputs)
    return out.astype(np.float32)
